# revision 52
# baseline (speedup 1.0000x reference)
"""Trainium2 Bass kernel for BertClassifierv4 (ragged premise/hypothesis classifier).

Strategy: pure data parallelism. 32 samples are sharded 4-per-core across 8
NeuronCores; all weights are replicated. Host-side numpy does the cheap
index-derived preprocessing (span masks, mean weights, head-padded weight
layouts); the device kernel does all the heavy lifting.

Device-side layout tricks:
  * hsT (bf16, [H, S]) is the canonical operand for every hs @ W matmul
    (PE contracts over partitions).
  * Q/K/V weights are padded per-head from 96 -> 128 so every head lives in
    its own partition tile; row 96 of Q is forced to 1.0 (via bias) and row 96
    of K is overwritten with the premise -1e9 mask, so the scores matmul
    produces masked scores directly in PSUM.
  * Softmax reductions over queries use matmuls with a broadcast [128,1]
    row-scale as lhsT, producing the weighted key-combination already
    broadcast across partitions; a fused DVE tensor_tensor_reduce against
    V^T then yields per-head context columns.
  * All tiny heads (feature extractor, diff/attn/align heads, classifier)
    run once per core batched over the 4 samples.
"""

import os
import sys

import numpy as np

if "/opt/trn_rl_repo" not in sys.path:
    sys.path.insert(0, "/opt/trn_rl_repo")

import ml_dtypes

import concourse.bass as bass
import concourse.bacc as bacc
import concourse.tile as tile
import concourse.tile_sem_assignment as _tsa

# DMA-completion semaphore lanes Tile round-robins over. The historical cap of
# 2 serialized DMA issue (each dma_start waited on the lane's previous user);
# with DMAs now merged into ~35 large transfers the wait-budget pressure that
# motivated the cap is gone.
_tsa.NUM_HWDGE_SEMS = 8
from concourse import mybir
from concourse.bass_utils import run_bass_kernel_spmd
from concourse.masks import make_identity

# Problem constants (hardcoded; kernel.py must be self-contained).
B, S, H = 32, 512, 768
NH, HD = 8, 96
NCLS = 3
SEP = 102
NEG = -1.0e9
NCORES = 8
BL = B // NCORES  # samples per core
HP = 128  # padded head width
HPAD = NH * HP  # 1024
KC = H // 128  # 6 contraction chunks for H
KL = 256  # premise/key range (s1 <= 255)
SQ = S // 128  # 4 seq partition tiles

F32 = mybir.dt.float32
F32R = mybir.dt.float32r
BF16 = mybir.dt.bfloat16

# brow offsets
_BOFF = {}
_off = 0
for _name, _n in [
    ("fe_b1", 512),
    ("fe_b2", 128),
    ("dp_b", 128),
    ("ap_b", 128),
    ("al_b12", 128),
    ("cl_b1", 64),
    ("cl_b2", NCLS),
]:
    _BOFF[_name] = (_off, _n)
    _off += _n
BROW_N = _off


def _build_bass():
    nc = bacc.Bacc(
        "TRN2",
        name="bert_cls_v4",
        num_devices=NCORES,
        use_seq_codegen=os.environ.get("BERT_SEQCG", "0") == "1",
    )

    def din(name, shape, dt):
        return nc.dram_tensor(name, shape, dt, kind="ExternalInput")

    d_hs = din("hs", [BL, S, H], BF16)
    d_hst = din("hst", [BL, H, S], BF16)
    d_wvec = din("wvec", [BL, S, 8], F32)
    d_rows = din("rows", [BL, KL + NH * KL + 2 * S], BF16)
    d_wq = din("wq", [H, HPAD], BF16)
    d_wk = din("wk", [H, HPAD], BF16)
    d_wv = din("wv", [H, HPAD], BF16)
    d_qb = din("qb", [128, NH], F32)
    d_kb = din("kb", [128, NH], F32)
    d_vb = din("vb", [128, NH], F32)
    d_woap = din("woap", [HPAD, 128], BF16)
    d_few1 = din("few1", [H, 512], BF16)
    d_few2 = din("few2", [512, 128], BF16)
    d_dpw = din("dpw", [H, 128], BF16)
    d_alw12 = din("alw12", [2 * H, 128], BF16)
    d_clw1 = din("clw1", [512, 64], BF16)
    d_clw2 = din("clw2", [64, NCLS], BF16)
    d_brow = din("brow", [1, BROW_N], F32)
    d_out = nc.dram_tensor("out", [BL, NCLS], F32, kind="ExternalOutput")

    AF = mybir.ActivationFunctionType
    OP = mybir.AluOpType
    AX = mybir.AxisListType

    with tile.TileContext(nc) as tc:
        with (
            tc.tile_pool(name="consts", bufs=1) as consts,
            tc.tile_pool(name="reps", bufs=1) as reps,
        ):
            # ---- resident weights ----
            def load_merged(pool, dram, rows, cols, dt, tag, eng, nsplit):
                all_t = pool.tile([128, nsplit * cols], dt, tag=tag, name=tag)
                eng.dma_start(
                    all_t[:].rearrange("p (k c) -> p k c", k=nsplit),
                    dram[:, :].rearrange("(k p) c -> p k c", p=128),
                )
                return all_t, [all_t[:, cols * k : cols * (k + 1)] for k in range(nsplit)]

            _, wq_sb = load_merged(consts, d_wq, H, HPAD, BF16, "wqa", nc.sync, KC)
            _, wk_sb = load_merged(consts, d_wk, H, HPAD, BF16, "wka", nc.sync, KC)
            _, wv_sb = load_merged(consts, d_wv, H, HPAD, BF16, "wva", nc.gpsimd, KC)
            qb_sb = consts.tile([128, NH], F32, tag="qb", name="qb")
            kb_sb = consts.tile([128, NH], F32, tag="kb", name="kb")
            vb_sb = consts.tile([128, NH], F32, tag="vb", name="vb")
            nc.sync.dma_start(qb_sb[:], d_qb[:, :])
            nc.sync.dma_start(kb_sb[:], d_kb[:, :])
            nc.sync.dma_start(vb_sb[:], d_vb[:, :])
            brow_sb = consts.tile([1, BROW_N], F32, tag="browf", name="browf")
            nc.sync.dma_start(brow_sb[:], d_brow[:, :])
            brow_bf = consts.tile([1, BROW_N], BF16, tag="browb", name="browb")
            nc.vector.tensor_copy(brow_bf[:], brow_sb[:])
            ones1_bf = consts.tile([1, 128], BF16, tag="ones1b", name="ones1b")
            nc.vector.memset(ones1_bf[:], 1.0)
            ones4_f = consts.tile([1, 4], F32, tag="ones4f", name="ones4f")
            nc.vector.memset(ones4_f[:], 1.0)
            ones4_b = consts.tile([1, 4], BF16, tag="ones4b", name="ones4b")
            nc.vector.memset(ones4_b[:], 1.0)
            ident4 = consts.tile([4, 4], F32, tag="id4", name="id4")
            make_identity(nc, ident4[:])

            # ---- persistent per-core representation columns ----
            ATT = reps.tile([128, NH * BL], BF16, tag="ATT", name="ATT")  # col BL*h+i
            ALC = reps.tile([128, 12 * BL], BF16, tag="ALC", name="ALC")  # col BL*c+i
            XFE = reps.tile([128, KC * BL], BF16, tag="XFE", name="XFE")  # col BL*j+i
            SDT = reps.tile([128, KC * BL], BF16, tag="SDT", name="SDT")
            MX = reps.tile([128, KC * BL], F32, tag="MX", name="MX")

            with (
                tc.tile_pool(name="sin", bufs=2) as sin,
                tc.tile_pool(name="sqkv", bufs=3) as sqkv,
                tc.tile_pool(name="sp", bufs=6) as sp,
                tc.tile_pool(name="ssc", bufs=2) as ssc,
                tc.tile_pool(name="pqkv", bufs=1, space="PSUM") as pqkv,
                tc.tile_pool(name="ppss", bufs=2, space="PSUM") as ppss,
                tc.tile_pool(name="ppsw", bufs=1, space="PSUM") as ppsw,
                tc.tile_pool(name="psim", bufs=2, space="PSUM") as psim,
                tc.tile_pool(name="psmall", bufs=1, space="PSUM") as psmall,
            ):
                for i in range(BL):
                    # ---------- loads ----------
                    hs_all = sin.tile([128, SQ * H], BF16, tag="hsa", name="hsa")
                    nc.gpsimd.dma_start(
                        hs_all[:].rearrange("p (c h) -> p c h", c=SQ),
                        d_hs[i, :, :].rearrange("(c p) h -> p c h", p=128),
                    )
                    hs_t = [hs_all[:, H * c : H * (c + 1)] for c in range(SQ)]
                    hst_all = sin.tile([128, KC * S], BF16, tag="hsta", name="hsta")
                    nc.gpsimd.dma_start(
                        hst_all[:, 0 : 3 * S].rearrange("p (k s) -> p k s", k=3),
                        d_hst[i, 0:384, :].rearrange("(k p) s -> p k s", p=128),
                    )
                    nc.gpsimd.dma_start(
                        hst_all[:, 3 * S : 6 * S].rearrange("p (k s) -> p k s", k=3),
                        d_hst[i, 384:768, :].rearrange("(k p) s -> p k s", p=128),
                    )
                    hst_t = [hst_all[:, S * k : S * (k + 1)] for k in range(KC)]
                    rhs6_all = sin.tile([128, SQ * 8], F32, tag="rhs6a", name="rhs6a")
                    nc.sync.dma_start(
                        rhs6_all[:].rearrange("p (c w) -> p c w", c=SQ),
                        d_wvec[i, :, :].rearrange("(c p) w -> p c w", p=128),
                    )
                    rhs6 = [rhs6_all[:, 8 * c : 8 * (c + 1)] for c in range(SQ)]
                    rows_sb = sin.tile([1, KL + NH * KL + 2 * S], BF16, tag="rows", name="rows")
                    nc.sync.dma_start(rows_sb[:], d_rows[i : i + 1, :])
                    pneg_sb = rows_sb[:, 0:KL]
                    pneg8_sb = rows_sb[:, KL : KL + NH * KL]
                    hneg_sb = rows_sb[:, KL + NH * KL : KL + NH * KL + S]
                    aneg_sb = rows_sb[:, KL + NH * KL + S : KL + NH * KL + 2 * S]

                    # ---------- QKV projections (head-padded) ----------
                    qpad = sqkv.tile([128, NH * S], BF16, tag="qpad", name="qpad")
                    kpad = sqkv.tile([128, NH * KL], BF16, tag="kpad", name="kpad")
                    vpad = sqkv.tile([128, NH * KL], BF16, tag="vpad", name="vpad")
                    # premise -1e9 mask into K row 96 (all heads) - independent
                    # of the K copies, which write only rows 0:96
                    nc.sync.dma_start(kpad[96:97, :], pneg8_sb)
                    for h in range(NH):
                        psq = pqkv.tile([128, S], F32, tag="psq", name="psq")
                        for k in range(KC):
                            nc.tensor.matmul(
                                psq[:],
                                lhsT=wq_sb[k][:, 128 * h : 128 * (h + 1)],
                                rhs=hst_t[k][:],
                                start=(k == 0),
                                stop=(k == KC - 1),
                            )
                        nc.scalar.activation(
                            qpad[0:97, S * h : S * (h + 1)],
                            psq[0:97, :],
                            AF.Identity,
                            bias=qb_sb[0:97, h : h + 1],
                        )
                        pskv = pqkv.tile([128, 2 * KL], F32, tag="pskv", name="pskv")
                        for k in range(KC):
                            nc.tensor.matmul(
                                pskv[:, 0:KL],
                                lhsT=wk_sb[k][:, 128 * h : 128 * (h + 1)],
                                rhs=hst_t[k][:, 0:KL],
                                start=(k == 0),
                                stop=(k == KC - 1),
                            )
                        nc.scalar.activation(
                            kpad[0:96, KL * h : KL * (h + 1)],
                            pskv[0:96, 0:KL],
                            AF.Identity,
                            bias=kb_sb[0:96, h : h + 1],
                        )
                        for k in range(KC):
                            nc.tensor.matmul(
                                pskv[:, KL : 2 * KL],
                                lhsT=wv_sb[k][:, 128 * h : 128 * (h + 1)],
                                rhs=hst_t[k][:, 0:KL],
                                start=(k == 0),
                                stop=(k == KC - 1),
                            )
                        nc.scalar.activation(
                            vpad[:, KL * h : KL * (h + 1)],
                            pskv[:, KL : 2 * KL],
                            AF.Identity,
                            bias=vb_sb[:, h : h + 1],
                        )

                    # ---------- attention: scores -> exp -> rowscale ----------
                    pat_t = []
                    rs_all = ssc.tile([128, 4 * NH], BF16, tag="rs", name="rs")
                    for t in range(SQ):
                        pat = sp.tile([128, NH * KL], BF16, tag="pat", name="pat")
                        for hh in range(4):
                            pss = ppss.tile([128, 2 * KL], F32, tag="pss", name="pss")
                            for h2 in range(2):
                                h = 2 * hh + h2
                                nc.tensor.matmul(
                                    pss[:, KL * h2 : KL * (h2 + 1)],
                                    lhsT=qpad[0:97, S * h + 128 * t : S * h + 128 * (t + 1)],
                                    rhs=kpad[0:97, KL * h : KL * (h + 1)],
                                    start=True,
                                    stop=True,
                                )
                            nc.scalar.activation(
                                pat[:, 2 * KL * hh : 2 * KL * (hh + 1)], pss[:], AF.Exp
                            )
                        den = ssc.tile([128, NH], F32, tag="den", name="den")
                        nc.vector.tensor_reduce(
                            den[:],
                            pat[:].rearrange("p (h k) -> p h k", h=NH),
                            axis=AX.X,
                            op=OP.add,
                        )
                        invd = ssc.tile([128, NH], F32, tag="invd", name="invd")
                        nc.vector.reciprocal(invd[:], den[:])
                        nc.vector.tensor_scalar(
                            rs_all[:, NH * t : NH * (t + 1)],
                            invd[:],
                            rhs6[t][:, 2:3],
                            None,
                            op0=OP.mult,
                        )
                        pat_t.append(pat)

                    # ---------- attention: weighted key-combination + context ----------
                    for h in range(NH):
                        psw = ppsw.tile([128, KL], F32, tag="psw", name="psw")
                        for t in range(SQ):
                            nc.tensor.matmul(
                                psw[:],
                                lhsT=rs_all[:, NH * t + h : NH * t + h + 1].to_broadcast(
                                    (128, 128)
                                ),
                                rhs=pat_t[t][:, KL * h : KL * (h + 1)],
                                start=(t == 0),
                                stop=(t == SQ - 1),
                            )
                        scr = ssc.tile([128, KL], BF16, tag="scr", name="scr")
                        nc.vector.scalar_tensor_tensor(
                            out=scr[:],
                            in0=vpad[:, KL * h : KL * (h + 1)],
                            scalar=1.0,
                            in1=psw[:],
                            op0=OP.mult,
                            op1=OP.mult,
                            accum_out=ATT[:, BL * h + i : BL * h + i + 1],
                        )

                    # ---------- alignment: p2h (A': rows 0:256, cols 0:512) ----------
                    psxw = psmall.tile([128, 64], F32, tag="psx", name="psx")
                    pswc = psxw[:, 48:56]
                    pa_t = []
                    dena = ssc.tile([128, 2], F32, tag="dena", name="dena")
                    for mt in range(2):
                        psa = psim.tile([128, S], F32, tag="pb", name="pb")
                        for k in range(KC):
                            nc.tensor.matmul(
                                psa[:],
                                lhsT=hst_t[k][:, 128 * mt : 128 * (mt + 1)],
                                rhs=hst_t[k][:],
                                start=(k == 0),
                                stop=False,
                            )
                        nc.tensor.matmul(
                            psa[:],
                            lhsT=ones1_bf[:],
                            rhs=hneg_sb,
                            start=False,
                            stop=True,
                        )
                        # row-max subtraction (sim diagonal ~ ||x||^2 ~ 768 would
                        # overflow exp otherwise)
                        nmax = ssc.tile([128, 1], F32, tag=f"nma{mt}", name=f"nma{mt}")
                        nc.vector.tensor_reduce(
                            nmax[:], psa[:], axis=AX.X, op=OP.max, negate=True
                        )
                        pa = sp.tile([128, S], BF16, tag="pa", name="pa")
                        nc.scalar.activation(
                            pa[:],
                            psa[:],
                            AF.Exp,
                            bias=nmax[:],
                            accum_out=dena[:, mt : mt + 1],
                        )
                        pa_t.append(pa)
                    invda = ssc.tile([128, 2], F32, tag="invda", name="invda")
                    nc.vector.reciprocal(invda[:], dena[:])
                    rsa = []
                    for mt in range(2):
                        r = ssc.tile([128, 1], BF16, tag=f"rsa{mt}", name=f"rsa{mt}")
                        nc.vector.tensor_scalar(
                            r[:],
                            invda[:, mt : mt + 1],
                            rhs6[mt][:, 1:2],
                            None,
                            op0=OP.mult,
                        )
                        rsa.append(r)
                    for tb in range(4):
                        for mt in range(2):
                            nc.tensor.matmul(
                                pswc[:, tb : tb + 1],
                                lhsT=pa_t[mt][:, 128 * tb : 128 * (tb + 1)],
                                rhs=rsa[mt][:],
                                start=(mt == 0),
                                stop=(mt == 1),
                            )

                    # ---------- alignment: h2p (B': rows 0:512, cols 0:256) ----------
                    pb_t = []
                    denb = ssc.tile([128, 4], F32, tag="denb", name="denb")
                    for mt in range(SQ):
                        psb = psim.tile([128, KL], F32, tag="pb", name="pb")
                        for k in range(KC):
                            nc.tensor.matmul(
                                psb[:],
                                lhsT=hst_t[k][:, 128 * mt : 128 * (mt + 1)],
                                rhs=hst_t[k][:, 0:KL],
                                start=(k == 0),
                                stop=False,
                            )
                        nc.tensor.matmul(
                            psb[:],
                            lhsT=ones1_bf[:],
                            rhs=pneg_sb,
                            start=False,
                            stop=True,
                        )

                        nmax = ssc.tile([128, 1], F32, tag=f"nmb{mt}", name=f"nmb{mt}")
                        nc.vector.tensor_reduce(
                            nmax[:], psb[:], axis=AX.X, op=OP.max, negate=True
                        )
                        pb = sp.tile([128, KL], BF16, tag="pbt", name="pbt")
                        nc.scalar.activation(
                            pb[:],
                            psb[:],
                            AF.Exp,
                            bias=nmax[:],
                            accum_out=denb[:, mt : mt + 1],
                        )
                        pb_t.append(pb)
                    invdb = ssc.tile([128, 4], F32, tag="invdb", name="invdb")
                    nc.vector.reciprocal(invdb[:], denb[:])
                    rsb = []
                    for mt in range(SQ):
                        r = ssc.tile([128, 1], BF16, tag=f"rsb{mt}", name=f"rsb{mt}")
                        nc.vector.tensor_scalar(
                            r[:],
                            invdb[:, mt : mt + 1],
                            rhs6[mt][:, 2:3],
                            None,
                            op0=OP.mult,
                        )
                        rsb.append(r)
                    for tb in range(2):
                        for mt in range(SQ):
                            nc.tensor.matmul(
                                pswc[:, 4 + tb : 5 + tb],
                                lhsT=pb_t[mt][:, 128 * tb : 128 * (tb + 1)],
                                rhs=rsb[mt][:],
                                start=(mt == 0),
                                stop=(mt == SQ - 1),
                            )
                    # move alignment combination vectors into the x6 rhs columns
                    for c in range(SQ):
                        nc.vector.tensor_copy(rhs6[c][:, 4:5], pswc[:, c : c + 1])
                    for c in range(2):
                        nc.vector.tensor_copy(rhs6[c][:, 5:6], pswc[:, 4 + c : 5 + c])
                    # bf16 copy of the 6 weight columns (matmul rhs must match
                    # the bf16 hs_t lhsT dtype)
                    rhs6b = []
                    for c in range(SQ):
                        t = ssc.tile([128, 6], BF16, tag=f"rhs6b{c}", name=f"rhs6b{c}")
                        nc.vector.tensor_copy(t[:], rhs6[c][:, 0:6])
                        rhs6b.append(t)

                    # ---------- masked max over sequence (per d-chunk) ----------
                    psneg = ppss.tile([128, 2 * KL], F32, tag="pss", name="pss")
                    nc.tensor.matmul(
                        psneg[:, 0:S], lhsT=ones1_bf[:], rhs=aneg_sb, start=True, stop=True
                    )
                    for k in range(KC):
                        scr2 = ssc.tile([128, S], F32, tag="scr2", name="scr2")
                        nc.vector.scalar_tensor_tensor(
                            out=scr2[:],
                            in0=hst_t[k][:],
                            scalar=0.0,
                            in1=psneg[:, 0:S],
                            op0=OP.add,
                            op1=OP.add,
                        )
                        nc.vector.tensor_reduce(
                            MX[:, BL * k + i : BL * k + i + 1],
                            scr2[:],
                            axis=AX.X,
                            op=OP.max,
                        )

                    # ---------- x6 matvec: [mean, prem-mean, hyp-mean, pooled, al1, al2] ----------
                    psx = psxw[:, 0:48]
                    for j in range(KC):
                        for c in range(SQ):
                            nc.tensor.matmul(
                                psxw[:, 8 * j : 8 * j + 6],
                                lhsT=hs_t[c][:, 128 * j : 128 * (j + 1)],
                                rhs=rhs6b[c][:],
                                start=(c == 0),
                                stop=(c == SQ - 1),
                            )
                    # stage PSUM x6 result through SBUF (DVE can read only one
                    # PSUM operand per instruction); copy only written columns
                    x6sb = ssc.tile([128, 36], F32, tag="x6sb", name="x6sb")
                    nc.vector.tensor_copy(
                        x6sb[:].rearrange("p (g c) -> p g c", g=KC),
                        psx.rearrange("p (g c) -> p g c", g=KC)[:, :, 0:6],
                    )
                    # strided views: cols i, i+BL, ... (count KC, step BL)
                    xfe_cols = XFE[:, i::BL]
                    sdt_cols = SDT[:, i::BL]
                    mx_cols = MX[:, i::BL]
                    mean_cols = x6sb[:, 0::6]
                    prem_cols = x6sb[:, 1::6]
                    hyp_cols = x6sb[:, 2::6]
                    pool_cols = x6sb[:, 3::6]
                    al1_cols = x6sb[:, 4::6]
                    al2_cols = x6sb[:, 5::6]
                    tmp6 = ssc.tile([128, KC], F32, tag="tmp6", name="tmp6")
                    nc.vector.tensor_add(tmp6[:], mean_cols, pool_cols)
                    nc.vector.tensor_add(xfe_cols, tmp6[:], mx_cols)
                    tmp7 = ssc.tile([128, KC], F32, tag="tmp7", name="tmp7")
                    nc.vector.tensor_sub(tmp7[:], prem_cols, hyp_cols)
                    nc.scalar.activation(sdt_cols, tmp7[:], AF.Abs)
                    alc1_cols = ALC[:, i : BL * KC : BL]
                    alc2_cols = ALC[:, BL * KC + i :: BL]
                    nc.vector.tensor_copy(alc1_cols, al1_cols)
                    nc.vector.tensor_copy(alc2_cols, al2_cols)

            # head-phase weights: issued after the sample loop so their DMAs
            # don't delay the first sample's input loads at startup
            _, woap_sb = load_merged(consts, d_woap, HPAD, 128, BF16, "woapa", nc.sync, 8)
            _, few1_sb = load_merged(consts, d_few1, H, 512, BF16, "few1a", nc.sync, KC)
            _, few2_sb = load_merged(consts, d_few2, 512, 128, BF16, "few2a", nc.sync, 4)
            _, dpw_sb = load_merged(consts, d_dpw, H, 128, BF16, "dpwa", nc.sync, KC)
            _, alw12_sb = load_merged(consts, d_alw12, 2 * H, 128, BF16, "alw12a", nc.sync, 12)
            _, clw1_sb = load_merged(consts, d_clw1, 512, 64, BF16, "clw1a", nc.sync, 4)
            clw2_sb = consts.tile([64, NCLS], BF16, tag="clw2", name="clw2")
            nc.sync.dma_start(clw2_sb[:], d_clw2[:, :])

            # ---------- per-core head phase (batched over BL samples) ----------
            with (
                tc.tile_pool(name="shead", bufs=2) as sh,
                tc.tile_pool(name="phead", bufs=1, space="PSUM") as ph,
                tc.tile_pool(name="ptr", bufs=2, space="PSUM") as ptr,
            ):
                def brow_f(name):
                    o, n = _BOFF[name]
                    return brow_sb[:, o : o + n]

                def brow_b(name):
                    o, n = _BOFF[name]
                    return brow_bf[:, o : o + n]

                # feature extractor first layer + layernorm
                psz1 = ph.tile([BL, 512], F32, tag="psz1", name="psz1")
                for j in range(KC):
                    nc.tensor.matmul(
                        psz1[:],
                        lhsT=XFE[:, BL * j : BL * (j + 1)],
                        rhs=few1_sb[j][:],
                        start=(j == 0),
                        stop=False,
                    )
                nc.tensor.matmul(
                    psz1[:], lhsT=ones4_b[:], rhs=brow_b("fe_b1"), start=False, stop=True
                )
                musum = sh.tile([BL, 1], F32, tag="musum", name="musum")
                nc.vector.tensor_reduce(musum[:], psz1[:], axis=AX.X, op=OP.add)
                mu = sh.tile([BL, 1], F32, tag="mu", name="mu")
                nc.vector.tensor_scalar(mu[:], musum[:], 1.0 / 512, None, op0=OP.mult)
                hc = sh.tile([BL, 512], F32, tag="hc", name="hc")
                nc.vector.tensor_scalar(hc[:], psz1[:], mu[:], None, op0=OP.subtract)
                sq = sh.tile([BL, 512], F32, tag="sq", name="sq")
                ssum = sh.tile([BL, 1], F32, tag="ssum", name="ssum")
                nc.vector.scalar_tensor_tensor(
                    out=sq[:],
                    in0=hc[:],
                    scalar=1.0,
                    in1=hc[:],
                    op0=OP.mult,
                    op1=OP.mult,
                    accum_out=ssum[:],
                )
                varv = sh.tile([BL, 1], F32, tag="varv", name="varv")
                nc.vector.tensor_scalar(
                    varv[:], ssum[:], 1.0 / 512, 1.0e-5, op0=OP.mult, op1=OP.add
                )
                lnv = sh.tile([BL, 1], F32, tag="lnv", name="lnv")
                nc.scalar.activation(lnv[:], varv[:], AF.Ln)
                rstd = sh.tile([BL, 1], F32, tag="rstd", name="rstd")
                nc.scalar.activation(rstd[:], lnv[:], AF.Exp, scale=-0.5)
                hn = sh.tile([BL, 512], F32, tag="hn", name="hn")
                nc.vector.tensor_scalar(hn[:], hc[:], rstd[:], None, op0=OP.mult)
                # transpose hn -> columns
                hnc = sh.tile([128, 4 * BL], BF16, tag="hnc", name="hnc")
                for c in range(4):
                    pt = ptr.tile([128, BL], F32, tag="pt", name="pt")
                    nc.tensor.transpose(pt[:], hn[:, 128 * c : 128 * (c + 1)], ident4[:])
                    nc.vector.tensor_copy(hnc[:, BL * c : BL * (c + 1)], pt[:])

                # Z assembly [BL, 512]: feat | diff | attn | align
                psZ = ph.tile([BL, 512], F32, tag="psZ", name="psZ")
                for c in range(4):
                    nc.tensor.matmul(
                        psZ[:, 0:128],
                        lhsT=hnc[:, BL * c : BL * (c + 1)],
                        rhs=few2_sb[c][:],
                        start=(c == 0),
                        stop=False,
                    )
                nc.tensor.matmul(
                    psZ[:, 0:128], lhsT=ones4_b[:], rhs=brow_b("fe_b2"),
                    start=False, stop=True,
                )
                for j in range(KC):
                    nc.tensor.matmul(
                        psZ[:, 128:256],
                        lhsT=SDT[:, BL * j : BL * (j + 1)],
                        rhs=dpw_sb[j][:],
                        start=(j == 0),
                        stop=False,
                    )
                nc.tensor.matmul(
                    psZ[:, 128:256], lhsT=ones4_b[:], rhs=brow_b("dp_b"),
                    start=False, stop=True,
                )
                for c in range(8):
                    nc.tensor.matmul(
                        psZ[:, 256:384],
                        lhsT=ATT[:, BL * c : BL * (c + 1)],
                        rhs=woap_sb[c][:],
                        start=(c == 0),
                        stop=False,
                    )
                nc.tensor.matmul(
                    psZ[:, 256:384], lhsT=ones4_b[:], rhs=brow_b("ap_b"),
                    start=False, stop=True,
                )
                for c in range(12):
                    nc.tensor.matmul(
                        psZ[:, 384:512],
                        lhsT=ALC[:, BL * c : BL * (c + 1)],
                        rhs=alw12_sb[c][:],
                        start=(c == 0),
                        stop=False,
                    )
                nc.tensor.matmul(
                    psZ[:, 384:512], lhsT=ones4_b[:], rhs=brow_b("al_b12"),
                    start=False, stop=True,
                )
                eZ = sh.tile([BL, 512], F32, tag="eZ", name="eZ")
                nc.scalar.activation(eZ[:], psZ[:], AF.Exp)
                tZ = sh.tile([BL, 512], F32, tag="tZ", name="tZ")
                nc.scalar.activation(tZ[:], eZ[:], AF.Tanh)
                comb = sh.tile([BL, 512], F32, tag="comb", name="comb")
                nc.vector.tensor_mul(comb[:], psZ[:], tZ[:])
                cbc = sh.tile([128, 4 * BL], BF16, tag="cbc", name="cbc")
                for c in range(4):
                    pt = ptr.tile([128, BL], F32, tag="pt", name="pt")
                    nc.tensor.transpose(pt[:], comb[:, 128 * c : 128 * (c + 1)], ident4[:])
                    nc.vector.tensor_copy(cbc[:, BL * c : BL * (c + 1)], pt[:])

                # classifier
                psz2 = ph.tile([BL, 64], F32, tag="psz2", name="psz2")
                for c in range(4):
                    nc.tensor.matmul(
                        psz2[:],
                        lhsT=cbc[:, BL * c : BL * (c + 1)],
                        rhs=clw1_sb[c][:],
                        start=(c == 0),
                        stop=False,
                    )
                nc.tensor.matmul(
                    psz2[:], lhsT=ones4_b[:], rhs=brow_b("cl_b1"), start=False, stop=True
                )
                eu = sh.tile([BL, 64], F32, tag="eu", name="eu")
                nc.scalar.activation(eu[:], psz2[:], AF.Exp)
                tu = sh.tile([BL, 64], F32, tag="tu", name="tu")
                nc.scalar.activation(tu[:], eu[:], AF.Tanh)
                uu = sh.tile([BL, 64], F32, tag="uu", name="uu")
                nc.vector.tensor_mul(uu[:], psz2[:], tu[:])
                ptu = ptr.tile([64, BL], F32, tag="pt", name="pt")
                nc.tensor.transpose(ptu[:], uu[:], ident4[:])
                uc = sh.tile([64, BL], BF16, tag="uc", name="uc")
                nc.vector.tensor_copy(uc[:], ptu[:])
                pslog = ph.tile([BL, NCLS], F32, tag="pslog", name="pslog")
                nc.tensor.matmul(pslog[:], lhsT=uc[:], rhs=clw2_sb[:], start=True, stop=False)
                nc.tensor.matmul(
                    pslog[:], lhsT=ones4_b[:], rhs=brow_b("cl_b2"), start=False, stop=True
                )
                out_sb = sh.tile([BL, NCLS], F32, tag="outsb", name="outsb")
                nc.vector.tensor_copy(out_sb[:], pslog[:])
                nc.sync.dma_start(d_out[:, :], out_sb[:])

    nc.compile()
    return nc


def _host_prep(inputs):
    """Compute per-core input maps from the full problem inputs."""
    f32 = np.float32
    bf16 = ml_dtypes.bfloat16
    hs = np.asarray(inputs["hidden_states"], dtype=f32)
    ids = np.asarray(inputs["input_ids"])
    am = np.asarray(inputs["attention_mask"]).astype(f32)

    sep = ids == SEP
    s1 = np.argmax(sep, axis=1)
    s2 = (S - 1) - np.argmax(sep[:, ::-1], axis=1)
    pos = np.arange(S)[None, :]
    prem = ((pos >= 1) & (pos < s1[:, None])).astype(f32)
    hyp = ((pos > s1[:, None]) & (pos < s2[:, None])).astype(f32)

    def wnorm(m):
        return m / np.clip(m.sum(1, keepdims=True), 1e-9, None)

    amw = wnorm(am)
    premw = wnorm(prem)
    hypw = wnorm(hyp)
    wvec = np.zeros((B, S, 8), dtype=f32)
    wvec[:, :, 0] = amw
    wvec[:, :, 1] = premw
    wvec[:, :, 2] = hypw
    wvec[:, 0, 3] = 1.0  # e0 -> pooled
    wvec[:, :, 6] = np.where(prem > 0, 0.0, NEG)  # premise row-mask for simA
    pneg = np.where(prem[:, :KL] > 0, 0.0, NEG).astype(bf16)
    hneg = np.where(hyp > 0, 0.0, NEG).astype(bf16)
    aneg = np.where(am > 0, 0.0, NEG).astype(bf16)

    hstf = np.ascontiguousarray(hs.transpose(0, 2, 1))
    hst = hstf.astype(bf16)
    hsb = hs.astype(bf16)

    def padw(w, b, scale=1.0, row96=0.0):
        w = np.asarray(w, dtype=f32) * scale
        b = np.asarray(b, dtype=f32) * scale
        wp = np.zeros((H, HPAD), dtype=f32)
        bp = np.zeros((128, NH), dtype=f32)
        for h in range(NH):
            wp[:, HP * h : HP * h + HD] = w[:, HD * h : HD * (h + 1)]
            bp[0:HD, h] = b[HD * h : HD * (h + 1)]
            bp[HD, h] = row96
        return wp.astype(bf16), bp

    isq = 1.0 / np.sqrt(np.float32(HD))
    wq_p, qb = padw(inputs["mha_wq"], inputs["mha_bq"], scale=isq, row96=1.0)
    wk_p, kb = padw(inputs["mha_wk"], inputs["mha_bk"], scale=1.0, row96=0.0)
    wv_p, vb = padw(inputs["mha_wv"], inputs["mha_bv"], scale=1.0, row96=0.0)

    wo = np.asarray(inputs["mha_wo"], dtype=f32)
    bo = np.asarray(inputs["mha_bo"], dtype=f32)
    ap_w = np.asarray(inputs["ap_w"], dtype=f32)
    ap_b = np.asarray(inputs["ap_b"], dtype=f32)
    woap768 = wo @ ap_w  # [768, 128]
    woap = np.zeros((HPAD, 128), dtype=f32)
    for h in range(NH):
        woap[HP * h : HP * h + HD, :] = woap768[HD * h : HD * (h + 1), :]
    ap_b_eff = bo @ ap_w + ap_b

    fe_w1 = np.asarray(inputs["fe_w1"], dtype=f32)
    fe_g = np.asarray(inputs["fe_g"], dtype=f32)
    fe_be = np.asarray(inputs["fe_be"], dtype=f32)
    fe_w2 = np.asarray(inputs["fe_w2"], dtype=f32)
    fe_b2 = np.asarray(inputs["fe_b2"], dtype=f32)
    # LN(h)*g + be then @ fe_w2 + fe_b2  ==  LNraw(h) @ (g*fe_w2) + (be@fe_w2 + fe_b2)
    few2 = fe_w2 * fe_g[:, None]
    fe_b2_eff = fe_be @ fe_w2 + fe_b2

    brow = np.zeros((1, BROW_N), dtype=f32)

    def setb(name, v):
        o, n = _BOFF[name]
        brow[0, o : o + n] = v

    setb("fe_b1", np.asarray(inputs["fe_b1"], dtype=f32))
    setb("fe_b2", fe_b2_eff)
    setb("dp_b", np.asarray(inputs["dp_b"], dtype=f32))
    setb("ap_b", ap_b_eff)
    al_w1 = np.asarray(inputs["al_w1"], dtype=f32)
    al_w2 = np.asarray(inputs["al_w2"], dtype=f32)
    al_b1 = np.asarray(inputs["al_b1"], dtype=f32)
    al_b2 = np.asarray(inputs["al_b2"], dtype=f32)
    setb("al_b12", al_b1 @ al_w2 + al_b2)
    setb("cl_b1", np.asarray(inputs["cl_b1"], dtype=f32))
    setb("cl_b2", np.asarray(inputs["cl_b2"], dtype=f32))

    shared = dict(
        wq=wq_p, wk=wk_p, wv=wv_p, qb=qb, kb=kb, vb=vb,
        woap=woap.astype(bf16),
        few1=fe_w1.astype(bf16),
        few2=few2.astype(bf16),
        dpw=np.asarray(inputs["dp_w"], dtype=f32).astype(bf16),
        alw12=(al_w1 @ al_w2).astype(bf16),
        clw1=np.asarray(inputs["cl_w1"], dtype=f32).astype(bf16),
        clw2=np.asarray(inputs["cl_w2"], dtype=f32).astype(bf16),
        brow=brow,
    )
    in_maps = []
    for core in range(NCORES):
        sl = slice(core * BL, (core + 1) * BL)
        m = dict(shared)
        m["hs"] = np.ascontiguousarray(hsb[sl])
        m["hst"] = np.ascontiguousarray(hst[sl])
        m["wvec"] = np.ascontiguousarray(wvec[sl])
        m["rows"] = np.ascontiguousarray(
            np.concatenate(
                [pneg[sl], np.tile(pneg[sl], (1, NH)), hneg[sl], aneg[sl]], axis=1
            )
        )
        in_maps.append(m)
    return in_maps


_NC_CACHE = {}


class _Exec:
    """Cached PJRT executable over the 8 axon-tunneled cores (mirrors
    bass2jax.run_bass_via_pjrt's multi-core path, but reusable so repeat
    calls don't re-trace/re-compile)."""

    def __init__(self):
        import jax
        import concourse.bass2jax as b2j
        from jax.experimental.shard_map import shard_map
        from jax.sharding import Mesh, PartitionSpec

        self.jax = jax
        self.b2j = b2j
        nc = _build_bass()
        self.nc = nc
        b2j.install_neuronx_cc_hook()
        in_names, out_names, out_avals = [], [], []
        partition_name = (
            nc.partition_id_tensor.name if nc.partition_id_tensor else None
        )
        for alloc in nc.m.functions[0].allocations:
            if not isinstance(alloc, mybir.MemoryLocationSet):
                continue
            name = alloc.memorylocations[0].name
            if alloc.kind == "ExternalInput":
                if name != partition_name:
                    in_names.append(name)
            elif alloc.kind == "ExternalOutput":
                out_names.append(name)
                out_avals.append(
                    jax.core.ShapedArray(
                        tuple(alloc.tensor_shape), mybir.dt.np(alloc.dtype)
                    )
                )
        self.in_names = list(in_names)
        self.out_names = list(out_names)
        self.out_avals = out_avals
        n_params = len(in_names)
        n_outs = len(out_avals)
        all_in_names = list(in_names) + list(out_names)
        if partition_name is not None:
            all_in_names.append(partition_name)
        donate = tuple(range(n_params, n_params + n_outs))

        def _body(*args):
            operands = list(args)
            if partition_name is not None:
                operands.append(b2j.partition_id_tensor())
            outs = b2j._bass_exec_p.bind(
                *operands,
                out_avals=tuple(out_avals),
                in_names=tuple(all_in_names),
                out_names=tuple(out_names),
                lowering_input_output_aliases=(),
                sim_require_finite=True,
                sim_require_nnan=True,
                nc=nc,
            )
            return tuple(outs)

        devices = jax.devices()[:NCORES]
        mesh = Mesh(np.asarray(devices), ("core",))
        in_specs = (PartitionSpec("core"),) * (n_params + n_outs)
        out_specs = (PartitionSpec("core"),) * n_outs
        self.sharded = jax.jit(
            shard_map(
                _body,
                mesh=mesh,
                in_specs=in_specs,
                out_specs=out_specs,
                check_rep=False,
            ),
            donate_argnums=donate,
            keep_unused=True,
        )

    def concat_inputs(self, in_maps):
        return [
            np.concatenate([m[name] for m in in_maps], axis=0)
            for name in self.in_names
        ]

    def zeros(self):
        return [
            np.zeros((NCORES * a.shape[0], *a.shape[1:]), a.dtype)
            for a in self.out_avals
        ]

    def run(self, concat_in):
        out_arrs = self.sharded(*concat_in, *self.zeros())
        return [np.asarray(o) for o in out_arrs]


def _get_exec():
    if "exec" not in _NC_CACHE:
        _NC_CACHE["exec"] = _Exec()
    return _NC_CACHE["exec"]


def _run_coresim(in_maps):
    """Fallback executor: run each core's shard through CoreSim (slow but
    exact) if the PJRT/hardware path is unavailable."""
    from concourse.bass_interp import CoreSim

    if "exec" in _NC_CACHE:
        nc = _NC_CACHE["exec"].nc
    elif "nc" in _NC_CACHE:
        nc = _NC_CACHE["nc"]
    else:
        nc = _NC_CACHE["nc"] = _build_bass()
    outs = []
    for m in in_maps:
        sim = CoreSim(nc, require_finite=False, require_nnan=False)
        for name, val in m.items():
            sim.tensor(name)[:] = val
        sim.simulate()
        outs.append(np.array(sim.tensor("out")))
    return np.concatenate(outs, axis=0)


def kernel(**inputs):
    in_maps = _host_prep(inputs)
    try:
        ex = _get_exec()
        concat_in = ex.concat_inputs(in_maps)
        outs = ex.run(concat_in)
        out = outs[ex.out_names.index("out")].reshape(B, NCLS)
    except Exception:
        out = _run_coresim(in_maps)
    return np.ascontiguousarray(out.astype(np.float32).reshape(B, NCLS))



# revision 56
# speedup vs baseline: 1.0153x; 1.0153x over previous
"""Trainium2 Bass kernel for BertClassifierv4 (ragged premise/hypothesis classifier).

Strategy: pure data parallelism. 32 samples are sharded 4-per-core across 8
NeuronCores; all weights are replicated. Host-side numpy does the cheap
index-derived preprocessing (span masks, mean weights, head-padded weight
layouts); the device kernel does all the heavy lifting.

Device-side layout tricks:
  * hsT (bf16, [H, S]) is the canonical operand for every hs @ W matmul
    (PE contracts over partitions).
  * Q/K/V weights are padded per-head from 96 -> 128 so every head lives in
    its own partition tile; row 96 of Q is forced to 1.0 (via bias) and row 96
    of K is overwritten with the premise -1e9 mask, so the scores matmul
    produces masked scores directly in PSUM.
  * Softmax reductions over queries use matmuls with a broadcast [128,1]
    row-scale as lhsT, producing the weighted key-combination already
    broadcast across partitions; a fused DVE tensor_tensor_reduce against
    V^T then yields per-head context columns.
  * All tiny heads (feature extractor, diff/attn/align heads, classifier)
    run once per core batched over the 4 samples.
"""

import os
import sys

import numpy as np

if "/opt/trn_rl_repo" not in sys.path:
    sys.path.insert(0, "/opt/trn_rl_repo")

import ml_dtypes

import concourse.bass as bass
import concourse.bacc as bacc
import concourse.tile as tile
import concourse.tile_sem_assignment as _tsa

# DMA-completion semaphore lanes Tile round-robins over. The historical cap of
# 2 serialized DMA issue (each dma_start waited on the lane's previous user);
# with DMAs now merged into ~35 large transfers the wait-budget pressure that
# motivated the cap is gone.
_tsa.NUM_HWDGE_SEMS = 8
from concourse import mybir
from concourse.bass_utils import run_bass_kernel_spmd
from concourse.masks import make_identity

# Problem constants (hardcoded; kernel.py must be self-contained).
B, S, H = 32, 512, 768
NH, HD = 8, 96
NCLS = 3
SEP = 102
NEG = -1.0e9
NCORES = 8
BL = B // NCORES  # samples per core
HP = 128  # padded head width
HPAD = NH * HP  # 1024
KC = H // 128  # 6 contraction chunks for H
KL = 256  # premise/key range (s1 <= 255)
SQ = S // 128  # 4 seq partition tiles

F32 = mybir.dt.float32
F32R = mybir.dt.float32r
BF16 = mybir.dt.bfloat16

# brow offsets
_BOFF = {}
_off = 0
for _name, _n in [
    ("fe_b1", 512),
    ("fe_b2", 128),
    ("dp_b", 128),
    ("ap_b", 128),
    ("al_b12", 128),
    ("cl_b1", 64),
    ("cl_b2", NCLS),
]:
    _BOFF[_name] = (_off, _n)
    _off += _n
BROW_N = _off


def _build_bass():
    nc = bacc.Bacc(
        "TRN2",
        name="bert_cls_v4",
        num_devices=NCORES,
        use_seq_codegen=os.environ.get("BERT_SEQCG", "0") == "1",
    )

    def din(name, shape, dt):
        return nc.dram_tensor(name, shape, dt, kind="ExternalInput")

    d_hs = din("hs", [BL, S, H], BF16)
    d_hst = din("hst", [BL, H, S], BF16)
    d_wvec = din("wvec", [BL, S, 8], F32)
    d_rows = din("rows", [BL, KL + NH * KL + 2 * S], BF16)
    d_wq = din("wq", [H, HPAD], BF16)
    d_wk = din("wk", [H, HPAD], BF16)
    d_wv = din("wv", [H, HPAD], BF16)
    d_qb = din("qb", [128, NH], F32)
    d_kb = din("kb", [128, NH], F32)
    d_vb = din("vb", [128, NH], F32)
    d_woap = din("woap", [HPAD, 128], BF16)
    d_few1 = din("few1", [H, 512], BF16)
    d_few2 = din("few2", [512, 128], BF16)
    d_dpw = din("dpw", [H, 128], BF16)
    d_alw12 = din("alw12", [2 * H, 128], BF16)
    d_clw1 = din("clw1", [512, 64], BF16)
    d_clw2 = din("clw2", [64, NCLS], BF16)
    d_brow = din("brow", [1, BROW_N], F32)
    d_out = nc.dram_tensor("out", [BL, NCLS], F32, kind="ExternalOutput")

    AF = mybir.ActivationFunctionType
    OP = mybir.AluOpType
    AX = mybir.AxisListType

    with tile.TileContext(nc) as tc:
        with (
            tc.tile_pool(name="consts", bufs=1) as consts,
            tc.tile_pool(name="reps", bufs=1) as reps,
        ):
            # ---- resident weights ----
            def load_merged(pool, dram, rows, cols, dt, tag, eng, nsplit):
                all_t = pool.tile([128, nsplit * cols], dt, tag=tag, name=tag)
                eng.dma_start(
                    all_t[:].rearrange("p (k c) -> p k c", k=nsplit),
                    dram[:, :].rearrange("(k p) c -> p k c", p=128),
                )
                return all_t, [all_t[:, cols * k : cols * (k + 1)] for k in range(nsplit)]

            _, wq_sb = load_merged(consts, d_wq, H, HPAD, BF16, "wqa", nc.sync, KC)
            _, wk_sb = load_merged(consts, d_wk, H, HPAD, BF16, "wka", nc.sync, KC)
            _, wv_sb = load_merged(consts, d_wv, H, HPAD, BF16, "wva", nc.scalar, KC)
            qb_sb = consts.tile([128, NH], F32, tag="qb", name="qb")
            kb_sb = consts.tile([128, NH], F32, tag="kb", name="kb")
            vb_sb = consts.tile([128, NH], F32, tag="vb", name="vb")
            nc.sync.dma_start(qb_sb[:], d_qb[:, :])
            nc.sync.dma_start(kb_sb[:], d_kb[:, :])
            nc.sync.dma_start(vb_sb[:], d_vb[:, :])
            brow_sb = consts.tile([1, BROW_N], F32, tag="browf", name="browf")
            nc.sync.dma_start(brow_sb[:], d_brow[:, :])
            brow_bf = consts.tile([1, BROW_N], BF16, tag="browb", name="browb")
            nc.vector.tensor_copy(brow_bf[:], brow_sb[:])
            ones1_bf = consts.tile([1, 128], BF16, tag="ones1b", name="ones1b")
            nc.vector.memset(ones1_bf[:], 1.0)
            ones4_f = consts.tile([1, 4], F32, tag="ones4f", name="ones4f")
            nc.vector.memset(ones4_f[:], 1.0)
            ones4_b = consts.tile([1, 4], BF16, tag="ones4b", name="ones4b")
            nc.vector.memset(ones4_b[:], 1.0)
            ident4 = consts.tile([4, 4], F32, tag="id4", name="id4")
            make_identity(nc, ident4[:])

            # ---- persistent per-core representation columns ----
            ATT = reps.tile([128, NH * BL], BF16, tag="ATT", name="ATT")  # col BL*h+i
            ALC = reps.tile([128, 12 * BL], BF16, tag="ALC", name="ALC")  # col BL*c+i
            XFE = reps.tile([128, KC * BL], BF16, tag="XFE", name="XFE")  # col BL*j+i
            SDT = reps.tile([128, KC * BL], BF16, tag="SDT", name="SDT")
            MX = reps.tile([128, KC * BL], F32, tag="MX", name="MX")

            with (
                tc.tile_pool(name="sin", bufs=2) as sin,
                tc.tile_pool(name="sqkv", bufs=3) as sqkv,
                tc.tile_pool(name="sp", bufs=6) as sp,
                tc.tile_pool(name="ssc", bufs=2) as ssc,
                tc.tile_pool(name="pqkv", bufs=1, space="PSUM") as pqkv,
                tc.tile_pool(name="ppss", bufs=2, space="PSUM") as ppss,
                tc.tile_pool(name="ppsw", bufs=1, space="PSUM") as ppsw,
                tc.tile_pool(name="psim", bufs=2, space="PSUM") as psim,
                tc.tile_pool(name="psmall", bufs=1, space="PSUM") as psmall,
            ):
                for i in range(BL):
                    # ---------- loads ----------
                    hst_all = sin.tile([128, KC * S], BF16, tag="hsta", name="hsta")
                    nc.gpsimd.dma_start(
                        hst_all[:, 0 : 3 * S].rearrange("p (k s) -> p k s", k=3),
                        d_hst[i, 0:384, :].rearrange("(k p) s -> p k s", p=128),
                    )
                    nc.gpsimd.dma_start(
                        hst_all[:, 3 * S : 6 * S].rearrange("p (k s) -> p k s", k=3),
                        d_hst[i, 384:768, :].rearrange("(k p) s -> p k s", p=128),
                    )
                    hst_t = [hst_all[:, S * k : S * (k + 1)] for k in range(KC)]
                    hs_all = sin.tile([128, SQ * H], BF16, tag="hsa", name="hsa")
                    nc.gpsimd.dma_start(
                        hs_all[:].rearrange("p (c h) -> p c h", c=SQ),
                        d_hs[i, :, :].rearrange("(c p) h -> p c h", p=128),
                    )
                    hs_t = [hs_all[:, H * c : H * (c + 1)] for c in range(SQ)]
                    rhs6_all = sin.tile([128, SQ * 8], F32, tag="rhs6a", name="rhs6a")
                    nc.sync.dma_start(
                        rhs6_all[:].rearrange("p (c w) -> p c w", c=SQ),
                        d_wvec[i, :, :].rearrange("(c p) w -> p c w", p=128),
                    )
                    rhs6 = [rhs6_all[:, 8 * c : 8 * (c + 1)] for c in range(SQ)]
                    rows_sb = sin.tile([1, KL + NH * KL + 2 * S], BF16, tag="rows", name="rows")
                    nc.sync.dma_start(rows_sb[:], d_rows[i : i + 1, :])
                    pneg_sb = rows_sb[:, 0:KL]
                    pneg8_sb = rows_sb[:, KL : KL + NH * KL]
                    hneg_sb = rows_sb[:, KL + NH * KL : KL + NH * KL + S]
                    aneg_sb = rows_sb[:, KL + NH * KL + S : KL + NH * KL + 2 * S]

                    # ---------- QKV projections (head-padded) ----------
                    qpad = sqkv.tile([128, NH * S], BF16, tag="qpad", name="qpad")
                    kpad = sqkv.tile([128, NH * KL], BF16, tag="kpad", name="kpad")
                    vpad = sqkv.tile([128, NH * KL], BF16, tag="vpad", name="vpad")
                    # premise -1e9 mask into K row 96 (all heads) - independent
                    # of the K copies, which write only rows 0:96
                    nc.sync.dma_start(kpad[96:97, :], pneg8_sb)
                    for h in range(NH):
                        psq = pqkv.tile([128, S], F32, tag="psq", name="psq")
                        for k in range(KC):
                            nc.tensor.matmul(
                                psq[:],
                                lhsT=wq_sb[k][:, 128 * h : 128 * (h + 1)],
                                rhs=hst_t[k][:],
                                start=(k == 0),
                                stop=(k == KC - 1),
                            )
                        nc.scalar.activation(
                            qpad[0:97, S * h : S * (h + 1)],
                            psq[0:97, :],
                            AF.Identity,
                            bias=qb_sb[0:97, h : h + 1],
                        )
                        pskv = pqkv.tile([128, 2 * KL], F32, tag="pskv", name="pskv")
                        for k in range(KC):
                            nc.tensor.matmul(
                                pskv[:, 0:KL],
                                lhsT=wk_sb[k][:, 128 * h : 128 * (h + 1)],
                                rhs=hst_t[k][:, 0:KL],
                                start=(k == 0),
                                stop=(k == KC - 1),
                            )
                        nc.scalar.activation(
                            kpad[0:96, KL * h : KL * (h + 1)],
                            pskv[0:96, 0:KL],
                            AF.Identity,
                            bias=kb_sb[0:96, h : h + 1],
                        )
                        for k in range(KC):
                            nc.tensor.matmul(
                                pskv[:, KL : 2 * KL],
                                lhsT=wv_sb[k][:, 128 * h : 128 * (h + 1)],
                                rhs=hst_t[k][:, 0:KL],
                                start=(k == 0),
                                stop=(k == KC - 1),
                            )
                        nc.scalar.activation(
                            vpad[:, KL * h : KL * (h + 1)],
                            pskv[:, KL : 2 * KL],
                            AF.Identity,
                            bias=vb_sb[:, h : h + 1],
                        )

                    # ---------- attention: scores -> exp -> rowscale ----------
                    pat_t = []
                    rs_all = ssc.tile([128, 4 * NH], BF16, tag="rs", name="rs")
                    for t in range(SQ):
                        pat = sp.tile([128, NH * KL], BF16, tag="pat", name="pat")
                        for hh in range(4):
                            pss = ppss.tile([128, 2 * KL], F32, tag="pss", name="pss")
                            for h2 in range(2):
                                h = 2 * hh + h2
                                nc.tensor.matmul(
                                    pss[:, KL * h2 : KL * (h2 + 1)],
                                    lhsT=qpad[0:97, S * h + 128 * t : S * h + 128 * (t + 1)],
                                    rhs=kpad[0:97, KL * h : KL * (h + 1)],
                                    start=True,
                                    stop=True,
                                )
                            nc.scalar.activation(
                                pat[:, 2 * KL * hh : 2 * KL * (hh + 1)], pss[:], AF.Exp
                            )
                        den = ssc.tile([128, NH], F32, tag="den", name="den")
                        nc.vector.tensor_reduce(
                            den[:],
                            pat[:].rearrange("p (h k) -> p h k", h=NH),
                            axis=AX.X,
                            op=OP.add,
                        )
                        invd = ssc.tile([128, NH], F32, tag="invd", name="invd")
                        nc.vector.reciprocal(invd[:], den[:])
                        nc.vector.tensor_scalar(
                            rs_all[:, NH * t : NH * (t + 1)],
                            invd[:],
                            rhs6[t][:, 2:3],
                            None,
                            op0=OP.mult,
                        )
                        pat_t.append(pat)

                    # ---------- attention: weighted key-combination + context ----------
                    for h in range(NH):
                        psw = ppsw.tile([128, KL], F32, tag="psw", name="psw")
                        for t in range(SQ):
                            nc.tensor.matmul(
                                psw[:],
                                lhsT=rs_all[:, NH * t + h : NH * t + h + 1].to_broadcast(
                                    (128, 128)
                                ),
                                rhs=pat_t[t][:, KL * h : KL * (h + 1)],
                                start=(t == 0),
                                stop=(t == SQ - 1),
                            )
                        scr = ssc.tile([128, KL], BF16, tag="scr", name="scr")
                        nc.vector.scalar_tensor_tensor(
                            out=scr[:],
                            in0=vpad[:, KL * h : KL * (h + 1)],
                            scalar=1.0,
                            in1=psw[:],
                            op0=OP.mult,
                            op1=OP.mult,
                            accum_out=ATT[:, BL * h + i : BL * h + i + 1],
                        )

                    # ---------- alignment: p2h (A': rows 0:256, cols 0:512) ----------
                    psxw = psmall.tile([128, 64], F32, tag="psx", name="psx")
                    pswc = psxw[:, 48:56]
                    pa_t = []
                    dena = ssc.tile([128, 2], F32, tag="dena", name="dena")
                    for mt in range(2):
                        psa = psim.tile([128, S], F32, tag="pb", name="pb")
                        for k in range(KC):
                            nc.tensor.matmul(
                                psa[:],
                                lhsT=hst_t[k][:, 128 * mt : 128 * (mt + 1)],
                                rhs=hst_t[k][:],
                                start=(k == 0),
                                stop=False,
                            )
                        nc.tensor.matmul(
                            psa[:],
                            lhsT=ones1_bf[:],
                            rhs=hneg_sb,
                            start=False,
                            stop=True,
                        )
                        # row-max subtraction (sim diagonal ~ ||x||^2 ~ 768 would
                        # overflow exp otherwise)
                        nmax = ssc.tile([128, 1], F32, tag=f"nma{mt}", name=f"nma{mt}")
                        nc.vector.tensor_reduce(
                            nmax[:], psa[:], axis=AX.X, op=OP.max, negate=True
                        )
                        pa = sp.tile([128, S], BF16, tag="pa", name="pa")
                        nc.scalar.activation(
                            pa[:],
                            psa[:],
                            AF.Exp,
                            bias=nmax[:],
                            accum_out=dena[:, mt : mt + 1],
                        )
                        pa_t.append(pa)
                    invda = ssc.tile([128, 2], F32, tag="invda", name="invda")
                    nc.vector.reciprocal(invda[:], dena[:])
                    rsa = []
                    for mt in range(2):
                        r = ssc.tile([128, 1], BF16, tag=f"rsa{mt}", name=f"rsa{mt}")
                        nc.vector.tensor_scalar(
                            r[:],
                            invda[:, mt : mt + 1],
                            rhs6[mt][:, 1:2],
                            None,
                            op0=OP.mult,
                        )
                        rsa.append(r)
                    for tb in range(4):
                        for mt in range(2):
                            nc.tensor.matmul(
                                pswc[:, tb : tb + 1],
                                lhsT=pa_t[mt][:, 128 * tb : 128 * (tb + 1)],
                                rhs=rsa[mt][:],
                                start=(mt == 0),
                                stop=(mt == 1),
                            )

                    # ---------- alignment: h2p (B': rows 0:512, cols 0:256) ----------
                    pb_t = []
                    denb = ssc.tile([128, 4], F32, tag="denb", name="denb")
                    for mt in range(SQ):
                        psb = psim.tile([128, KL], F32, tag="pb", name="pb")
                        for k in range(KC):
                            nc.tensor.matmul(
                                psb[:],
                                lhsT=hst_t[k][:, 128 * mt : 128 * (mt + 1)],
                                rhs=hst_t[k][:, 0:KL],
                                start=(k == 0),
                                stop=False,
                            )
                        nc.tensor.matmul(
                            psb[:],
                            lhsT=ones1_bf[:],
                            rhs=pneg_sb,
                            start=False,
                            stop=True,
                        )

                        nmax = ssc.tile([128, 1], F32, tag=f"nmb{mt}", name=f"nmb{mt}")
                        nc.vector.tensor_reduce(
                            nmax[:], psb[:], axis=AX.X, op=OP.max, negate=True
                        )
                        pb = sp.tile([128, KL], BF16, tag="pbt", name="pbt")
                        nc.scalar.activation(
                            pb[:],
                            psb[:],
                            AF.Exp,
                            bias=nmax[:],
                            accum_out=denb[:, mt : mt + 1],
                        )
                        pb_t.append(pb)
                    invdb = ssc.tile([128, 4], F32, tag="invdb", name="invdb")
                    nc.vector.reciprocal(invdb[:], denb[:])
                    rsb = []
                    for mt in range(SQ):
                        r = ssc.tile([128, 1], BF16, tag=f"rsb{mt}", name=f"rsb{mt}")
                        nc.vector.tensor_scalar(
                            r[:],
                            invdb[:, mt : mt + 1],
                            rhs6[mt][:, 2:3],
                            None,
                            op0=OP.mult,
                        )
                        rsb.append(r)
                    for tb in range(2):
                        for mt in range(SQ):
                            nc.tensor.matmul(
                                pswc[:, 4 + tb : 5 + tb],
                                lhsT=pb_t[mt][:, 128 * tb : 128 * (tb + 1)],
                                rhs=rsb[mt][:],
                                start=(mt == 0),
                                stop=(mt == SQ - 1),
                            )
                    # move alignment combination vectors into the x6 rhs columns
                    for c in range(SQ):
                        nc.vector.tensor_copy(rhs6[c][:, 4:5], pswc[:, c : c + 1])
                    for c in range(2):
                        nc.vector.tensor_copy(rhs6[c][:, 5:6], pswc[:, 4 + c : 5 + c])
                    # bf16 copy of the 6 weight columns (matmul rhs must match
                    # the bf16 hs_t lhsT dtype)
                    rhs6b = []
                    for c in range(SQ):
                        t = ssc.tile([128, 6], BF16, tag=f"rhs6b{c}", name=f"rhs6b{c}")
                        nc.vector.tensor_copy(t[:], rhs6[c][:, 0:6])
                        rhs6b.append(t)

                    # ---------- masked max over sequence (per d-chunk) ----------
                    psneg = ppss.tile([128, 2 * KL], F32, tag="pss", name="pss")
                    nc.tensor.matmul(
                        psneg[:, 0:S], lhsT=ones1_bf[:], rhs=aneg_sb, start=True, stop=True
                    )
                    for k in range(KC):
                        scr2 = ssc.tile([128, S], F32, tag="scr2", name="scr2")
                        nc.vector.scalar_tensor_tensor(
                            out=scr2[:],
                            in0=hst_t[k][:],
                            scalar=0.0,
                            in1=psneg[:, 0:S],
                            op0=OP.add,
                            op1=OP.add,
                        )
                        nc.vector.tensor_reduce(
                            MX[:, BL * k + i : BL * k + i + 1],
                            scr2[:],
                            axis=AX.X,
                            op=OP.max,
                        )

                    # ---------- x6 matvec: [mean, prem-mean, hyp-mean, pooled, al1, al2] ----------
                    psx = psxw[:, 0:48]
                    for j in range(KC):
                        for c in range(SQ):
                            nc.tensor.matmul(
                                psxw[:, 8 * j : 8 * j + 6],
                                lhsT=hs_t[c][:, 128 * j : 128 * (j + 1)],
                                rhs=rhs6b[c][:],
                                start=(c == 0),
                                stop=(c == SQ - 1),
                            )
                    # stage PSUM x6 result through SBUF (DVE can read only one
                    # PSUM operand per instruction); copy only written columns
                    x6sb = ssc.tile([128, 36], F32, tag="x6sb", name="x6sb")
                    nc.vector.tensor_copy(
                        x6sb[:].rearrange("p (g c) -> p g c", g=KC),
                        psx.rearrange("p (g c) -> p g c", g=KC)[:, :, 0:6],
                    )
                    # strided views: cols i, i+BL, ... (count KC, step BL)
                    xfe_cols = XFE[:, i::BL]
                    sdt_cols = SDT[:, i::BL]
                    mx_cols = MX[:, i::BL]
                    mean_cols = x6sb[:, 0::6]
                    prem_cols = x6sb[:, 1::6]
                    hyp_cols = x6sb[:, 2::6]
                    pool_cols = x6sb[:, 3::6]
                    al1_cols = x6sb[:, 4::6]
                    al2_cols = x6sb[:, 5::6]
                    tmp6 = ssc.tile([128, KC], F32, tag="tmp6", name="tmp6")
                    nc.vector.tensor_add(tmp6[:], mean_cols, pool_cols)
                    nc.vector.tensor_add(xfe_cols, tmp6[:], mx_cols)
                    tmp7 = ssc.tile([128, KC], F32, tag="tmp7", name="tmp7")
                    nc.vector.tensor_sub(tmp7[:], prem_cols, hyp_cols)
                    nc.scalar.activation(sdt_cols, tmp7[:], AF.Abs)
                    alc1_cols = ALC[:, i : BL * KC : BL]
                    alc2_cols = ALC[:, BL * KC + i :: BL]
                    nc.vector.tensor_copy(alc1_cols, al1_cols)
                    nc.vector.tensor_copy(alc2_cols, al2_cols)

            # head-phase weights: issued after the sample loop so their DMAs
            # don't delay the first sample's input loads at startup
            _, woap_sb = load_merged(consts, d_woap, HPAD, 128, BF16, "woapa", nc.sync, 8)
            _, few1_sb = load_merged(consts, d_few1, H, 512, BF16, "few1a", nc.sync, KC)
            _, few2_sb = load_merged(consts, d_few2, 512, 128, BF16, "few2a", nc.sync, 4)
            _, dpw_sb = load_merged(consts, d_dpw, H, 128, BF16, "dpwa", nc.sync, KC)
            _, alw12_sb = load_merged(consts, d_alw12, 2 * H, 128, BF16, "alw12a", nc.sync, 12)
            _, clw1_sb = load_merged(consts, d_clw1, 512, 64, BF16, "clw1a", nc.sync, 4)
            clw2_sb = consts.tile([64, NCLS], BF16, tag="clw2", name="clw2")
            nc.sync.dma_start(clw2_sb[:], d_clw2[:, :])

            # ---------- per-core head phase (batched over BL samples) ----------
            with (
                tc.tile_pool(name="shead", bufs=2) as sh,
                tc.tile_pool(name="phead", bufs=1, space="PSUM") as ph,
                tc.tile_pool(name="ptr", bufs=2, space="PSUM") as ptr,
            ):
                def brow_f(name):
                    o, n = _BOFF[name]
                    return brow_sb[:, o : o + n]

                def brow_b(name):
                    o, n = _BOFF[name]
                    return brow_bf[:, o : o + n]

                # Z assembly [BL, 512]: feat | diff | attn | align
                psZ = ph.tile([BL, 512], F32, tag="psZ", name="psZ")
                for j in range(KC):
                    nc.tensor.matmul(
                        psZ[:, 128:256],
                        lhsT=SDT[:, BL * j : BL * (j + 1)],
                        rhs=dpw_sb[j][:],
                        start=(j == 0),
                        stop=False,
                    )
                nc.tensor.matmul(
                    psZ[:, 128:256], lhsT=ones4_b[:], rhs=brow_b("dp_b"),
                    start=False, stop=True,
                )
                for c in range(8):
                    nc.tensor.matmul(
                        psZ[:, 256:384],
                        lhsT=ATT[:, BL * c : BL * (c + 1)],
                        rhs=woap_sb[c][:],
                        start=(c == 0),
                        stop=False,
                    )
                nc.tensor.matmul(
                    psZ[:, 256:384], lhsT=ones4_b[:], rhs=brow_b("ap_b"),
                    start=False, stop=True,
                )
                for c in range(12):
                    nc.tensor.matmul(
                        psZ[:, 384:512],
                        lhsT=ALC[:, BL * c : BL * (c + 1)],
                        rhs=alw12_sb[c][:],
                        start=(c == 0),
                        stop=False,
                    )
                nc.tensor.matmul(
                    psZ[:, 384:512], lhsT=ones4_b[:], rhs=brow_b("al_b12"),
                    start=False, stop=True,
                )
                # feature extractor first layer + layernorm
                psz1 = ph.tile([BL, 512], F32, tag="psz1", name="psz1")
                for j in range(KC):
                    nc.tensor.matmul(
                        psz1[:],
                        lhsT=XFE[:, BL * j : BL * (j + 1)],
                        rhs=few1_sb[j][:],
                        start=(j == 0),
                        stop=False,
                    )
                nc.tensor.matmul(
                    psz1[:], lhsT=ones4_b[:], rhs=brow_b("fe_b1"), start=False, stop=True
                )
                musum = sh.tile([BL, 1], F32, tag="musum", name="musum")
                nc.vector.tensor_reduce(musum[:], psz1[:], axis=AX.X, op=OP.add)
                mu = sh.tile([BL, 1], F32, tag="mu", name="mu")
                nc.vector.tensor_scalar(mu[:], musum[:], 1.0 / 512, None, op0=OP.mult)
                hc = sh.tile([BL, 512], F32, tag="hc", name="hc")
                nc.vector.tensor_scalar(hc[:], psz1[:], mu[:], None, op0=OP.subtract)
                sq = sh.tile([BL, 512], F32, tag="sq", name="sq")
                ssum = sh.tile([BL, 1], F32, tag="ssum", name="ssum")
                nc.vector.scalar_tensor_tensor(
                    out=sq[:],
                    in0=hc[:],
                    scalar=1.0,
                    in1=hc[:],
                    op0=OP.mult,
                    op1=OP.mult,
                    accum_out=ssum[:],
                )
                varv = sh.tile([BL, 1], F32, tag="varv", name="varv")
                nc.vector.tensor_scalar(
                    varv[:], ssum[:], 1.0 / 512, 1.0e-5, op0=OP.mult, op1=OP.add
                )
                lnv = sh.tile([BL, 1], F32, tag="lnv", name="lnv")
                nc.scalar.activation(lnv[:], varv[:], AF.Ln)
                rstd = sh.tile([BL, 1], F32, tag="rstd", name="rstd")
                nc.scalar.activation(rstd[:], lnv[:], AF.Exp, scale=-0.5)
                hn = sh.tile([BL, 512], F32, tag="hn", name="hn")
                nc.vector.tensor_scalar(hn[:], hc[:], rstd[:], None, op0=OP.mult)
                # transpose hn -> columns
                hnc = sh.tile([128, 4 * BL], BF16, tag="hnc", name="hnc")
                for c in range(4):
                    pt = ptr.tile([128, BL], F32, tag="pt", name="pt")
                    nc.tensor.transpose(pt[:], hn[:, 128 * c : 128 * (c + 1)], ident4[:])
                    nc.vector.tensor_copy(hnc[:, BL * c : BL * (c + 1)], pt[:])

                for c in range(4):
                    nc.tensor.matmul(
                        psZ[:, 0:128],
                        lhsT=hnc[:, BL * c : BL * (c + 1)],
                        rhs=few2_sb[c][:],
                        start=(c == 0),
                        stop=False,
                    )
                nc.tensor.matmul(
                    psZ[:, 0:128], lhsT=ones4_b[:], rhs=brow_b("fe_b2"),
                    start=False, stop=True,
                )
                eZ = sh.tile([BL, 512], F32, tag="eZ", name="eZ")
                nc.scalar.activation(eZ[:], psZ[:], AF.Exp)
                tZ = sh.tile([BL, 512], F32, tag="tZ", name="tZ")
                nc.scalar.activation(tZ[:], eZ[:], AF.Tanh)
                comb = sh.tile([BL, 512], F32, tag="comb", name="comb")
                nc.vector.tensor_mul(comb[:], psZ[:], tZ[:])
                cbc = sh.tile([128, 4 * BL], BF16, tag="cbc", name="cbc")
                for c in range(4):
                    pt = ptr.tile([128, BL], F32, tag="pt", name="pt")
                    nc.tensor.transpose(pt[:], comb[:, 128 * c : 128 * (c + 1)], ident4[:])
                    nc.vector.tensor_copy(cbc[:, BL * c : BL * (c + 1)], pt[:])

                # classifier
                psz2 = ph.tile([BL, 64], F32, tag="psz2", name="psz2")
                for c in range(4):
                    nc.tensor.matmul(
                        psz2[:],
                        lhsT=cbc[:, BL * c : BL * (c + 1)],
                        rhs=clw1_sb[c][:],
                        start=(c == 0),
                        stop=False,
                    )
                nc.tensor.matmul(
                    psz2[:], lhsT=ones4_b[:], rhs=brow_b("cl_b1"), start=False, stop=True
                )
                eu = sh.tile([BL, 64], F32, tag="eu", name="eu")
                nc.scalar.activation(eu[:], psz2[:], AF.Exp)
                tu = sh.tile([BL, 64], F32, tag="tu", name="tu")
                nc.scalar.activation(tu[:], eu[:], AF.Tanh)
                uu = sh.tile([BL, 64], F32, tag="uu", name="uu")
                nc.vector.tensor_mul(uu[:], psz2[:], tu[:])
                ptu = ptr.tile([64, BL], F32, tag="pt", name="pt")
                nc.tensor.transpose(ptu[:], uu[:], ident4[:])
                uc = sh.tile([64, BL], BF16, tag="uc", name="uc")
                nc.vector.tensor_copy(uc[:], ptu[:])
                pslog = ph.tile([BL, NCLS], F32, tag="pslog", name="pslog")
                nc.tensor.matmul(pslog[:], lhsT=uc[:], rhs=clw2_sb[:], start=True, stop=False)
                nc.tensor.matmul(
                    pslog[:], lhsT=ones4_b[:], rhs=brow_b("cl_b2"), start=False, stop=True
                )
                out_sb = sh.tile([BL, NCLS], F32, tag="outsb", name="outsb")
                nc.vector.tensor_copy(out_sb[:], pslog[:])
                nc.sync.dma_start(d_out[:, :], out_sb[:])

    nc.compile()
    return nc


def _host_prep(inputs):
    """Compute per-core input maps from the full problem inputs."""
    f32 = np.float32
    bf16 = ml_dtypes.bfloat16
    hs = np.asarray(inputs["hidden_states"], dtype=f32)
    ids = np.asarray(inputs["input_ids"])
    am = np.asarray(inputs["attention_mask"]).astype(f32)

    sep = ids == SEP
    s1 = np.argmax(sep, axis=1)
    s2 = (S - 1) - np.argmax(sep[:, ::-1], axis=1)
    pos = np.arange(S)[None, :]
    prem = ((pos >= 1) & (pos < s1[:, None])).astype(f32)
    hyp = ((pos > s1[:, None]) & (pos < s2[:, None])).astype(f32)

    def wnorm(m):
        return m / np.clip(m.sum(1, keepdims=True), 1e-9, None)

    amw = wnorm(am)
    premw = wnorm(prem)
    hypw = wnorm(hyp)
    wvec = np.zeros((B, S, 8), dtype=f32)
    wvec[:, :, 0] = amw
    wvec[:, :, 1] = premw
    wvec[:, :, 2] = hypw
    wvec[:, 0, 3] = 1.0  # e0 -> pooled
    wvec[:, :, 6] = np.where(prem > 0, 0.0, NEG)  # premise row-mask for simA
    pneg = np.where(prem[:, :KL] > 0, 0.0, NEG).astype(bf16)
    hneg = np.where(hyp > 0, 0.0, NEG).astype(bf16)
    aneg = np.where(am > 0, 0.0, NEG).astype(bf16)

    hstf = np.ascontiguousarray(hs.transpose(0, 2, 1))
    hst = hstf.astype(bf16)
    hsb = hs.astype(bf16)

    def padw(w, b, scale=1.0, row96=0.0):
        w = np.asarray(w, dtype=f32) * scale
        b = np.asarray(b, dtype=f32) * scale
        wp = np.zeros((H, HPAD), dtype=f32)
        bp = np.zeros((128, NH), dtype=f32)
        for h in range(NH):
            wp[:, HP * h : HP * h + HD] = w[:, HD * h : HD * (h + 1)]
            bp[0:HD, h] = b[HD * h : HD * (h + 1)]
            bp[HD, h] = row96
        return wp.astype(bf16), bp

    isq = 1.0 / np.sqrt(np.float32(HD))
    wq_p, qb = padw(inputs["mha_wq"], inputs["mha_bq"], scale=isq, row96=1.0)
    wk_p, kb = padw(inputs["mha_wk"], inputs["mha_bk"], scale=1.0, row96=0.0)
    wv_p, vb = padw(inputs["mha_wv"], inputs["mha_bv"], scale=1.0, row96=0.0)

    wo = np.asarray(inputs["mha_wo"], dtype=f32)
    bo = np.asarray(inputs["mha_bo"], dtype=f32)
    ap_w = np.asarray(inputs["ap_w"], dtype=f32)
    ap_b = np.asarray(inputs["ap_b"], dtype=f32)
    woap768 = wo @ ap_w  # [768, 128]
    woap = np.zeros((HPAD, 128), dtype=f32)
    for h in range(NH):
        woap[HP * h : HP * h + HD, :] = woap768[HD * h : HD * (h + 1), :]
    ap_b_eff = bo @ ap_w + ap_b

    fe_w1 = np.asarray(inputs["fe_w1"], dtype=f32)
    fe_g = np.asarray(inputs["fe_g"], dtype=f32)
    fe_be = np.asarray(inputs["fe_be"], dtype=f32)
    fe_w2 = np.asarray(inputs["fe_w2"], dtype=f32)
    fe_b2 = np.asarray(inputs["fe_b2"], dtype=f32)
    # LN(h)*g + be then @ fe_w2 + fe_b2  ==  LNraw(h) @ (g*fe_w2) + (be@fe_w2 + fe_b2)
    few2 = fe_w2 * fe_g[:, None]
    fe_b2_eff = fe_be @ fe_w2 + fe_b2

    brow = np.zeros((1, BROW_N), dtype=f32)

    def setb(name, v):
        o, n = _BOFF[name]
        brow[0, o : o + n] = v

    setb("fe_b1", np.asarray(inputs["fe_b1"], dtype=f32))
    setb("fe_b2", fe_b2_eff)
    setb("dp_b", np.asarray(inputs["dp_b"], dtype=f32))
    setb("ap_b", ap_b_eff)
    al_w1 = np.asarray(inputs["al_w1"], dtype=f32)
    al_w2 = np.asarray(inputs["al_w2"], dtype=f32)
    al_b1 = np.asarray(inputs["al_b1"], dtype=f32)
    al_b2 = np.asarray(inputs["al_b2"], dtype=f32)
    setb("al_b12", al_b1 @ al_w2 + al_b2)
    setb("cl_b1", np.asarray(inputs["cl_b1"], dtype=f32))
    setb("cl_b2", np.asarray(inputs["cl_b2"], dtype=f32))

    shared = dict(
        wq=wq_p, wk=wk_p, wv=wv_p, qb=qb, kb=kb, vb=vb,
        woap=woap.astype(bf16),
        few1=fe_w1.astype(bf16),
        few2=few2.astype(bf16),
        dpw=np.asarray(inputs["dp_w"], dtype=f32).astype(bf16),
        alw12=(al_w1 @ al_w2).astype(bf16),
        clw1=np.asarray(inputs["cl_w1"], dtype=f32).astype(bf16),
        clw2=np.asarray(inputs["cl_w2"], dtype=f32).astype(bf16),
        brow=brow,
    )
    in_maps = []
    for core in range(NCORES):
        sl = slice(core * BL, (core + 1) * BL)
        m = dict(shared)
        m["hs"] = np.ascontiguousarray(hsb[sl])
        m["hst"] = np.ascontiguousarray(hst[sl])
        m["wvec"] = np.ascontiguousarray(wvec[sl])
        m["rows"] = np.ascontiguousarray(
            np.concatenate(
                [pneg[sl], np.tile(pneg[sl], (1, NH)), hneg[sl], aneg[sl]], axis=1
            )
        )
        in_maps.append(m)
    return in_maps


_NC_CACHE = {}


class _Exec:
    """Cached PJRT executable over the 8 axon-tunneled cores (mirrors
    bass2jax.run_bass_via_pjrt's multi-core path, but reusable so repeat
    calls don't re-trace/re-compile)."""

    def __init__(self):
        import jax
        import concourse.bass2jax as b2j
        from jax.experimental.shard_map import shard_map
        from jax.sharding import Mesh, PartitionSpec

        self.jax = jax
        self.b2j = b2j
        nc = _build_bass()
        self.nc = nc
        b2j.install_neuronx_cc_hook()
        in_names, out_names, out_avals = [], [], []
        partition_name = (
            nc.partition_id_tensor.name if nc.partition_id_tensor else None
        )
        for alloc in nc.m.functions[0].allocations:
            if not isinstance(alloc, mybir.MemoryLocationSet):
                continue
            name = alloc.memorylocations[0].name
            if alloc.kind == "ExternalInput":
                if name != partition_name:
                    in_names.append(name)
            elif alloc.kind == "ExternalOutput":
                out_names.append(name)
                out_avals.append(
                    jax.core.ShapedArray(
                        tuple(alloc.tensor_shape), mybir.dt.np(alloc.dtype)
                    )
                )
        self.in_names = list(in_names)
        self.out_names = list(out_names)
        self.out_avals = out_avals
        n_params = len(in_names)
        n_outs = len(out_avals)
        all_in_names = list(in_names) + list(out_names)
        if partition_name is not None:
            all_in_names.append(partition_name)
        donate = tuple(range(n_params, n_params + n_outs))

        def _body(*args):
            operands = list(args)
            if partition_name is not None:
                operands.append(b2j.partition_id_tensor())
            outs = b2j._bass_exec_p.bind(
                *operands,
                out_avals=tuple(out_avals),
                in_names=tuple(all_in_names),
                out_names=tuple(out_names),
                lowering_input_output_aliases=(),
                sim_require_finite=True,
                sim_require_nnan=True,
                nc=nc,
            )
            return tuple(outs)

        devices = jax.devices()[:NCORES]
        mesh = Mesh(np.asarray(devices), ("core",))
        in_specs = (PartitionSpec("core"),) * (n_params + n_outs)
        out_specs = (PartitionSpec("core"),) * n_outs
        self.sharded = jax.jit(
            shard_map(
                _body,
                mesh=mesh,
                in_specs=in_specs,
                out_specs=out_specs,
                check_rep=False,
            ),
            donate_argnums=donate,
            keep_unused=True,
        )

    def concat_inputs(self, in_maps):
        return [
            np.concatenate([m[name] for m in in_maps], axis=0)
            for name in self.in_names
        ]

    def zeros(self):
        return [
            np.zeros((NCORES * a.shape[0], *a.shape[1:]), a.dtype)
            for a in self.out_avals
        ]

    def run(self, concat_in):
        out_arrs = self.sharded(*concat_in, *self.zeros())
        return [np.asarray(o) for o in out_arrs]


def _get_exec():
    if "exec" not in _NC_CACHE:
        _NC_CACHE["exec"] = _Exec()
    return _NC_CACHE["exec"]


def _run_coresim(in_maps):
    """Fallback executor: run each core's shard through CoreSim (slow but
    exact) if the PJRT/hardware path is unavailable."""
    from concourse.bass_interp import CoreSim

    if "exec" in _NC_CACHE:
        nc = _NC_CACHE["exec"].nc
    elif "nc" in _NC_CACHE:
        nc = _NC_CACHE["nc"]
    else:
        nc = _NC_CACHE["nc"] = _build_bass()
    outs = []
    for m in in_maps:
        sim = CoreSim(nc, require_finite=False, require_nnan=False)
        for name, val in m.items():
            sim.tensor(name)[:] = val
        sim.simulate()
        outs.append(np.array(sim.tensor("out")))
    return np.concatenate(outs, axis=0)


def kernel(**inputs):
    in_maps = _host_prep(inputs)
    try:
        ex = _get_exec()
        concat_in = ex.concat_inputs(in_maps)
        outs = ex.run(concat_in)
        out = outs[ex.out_names.index("out")].reshape(B, NCLS)
    except Exception:
        out = _run_coresim(in_maps)
    return np.ascontiguousarray(out.astype(np.float32).reshape(B, NCLS))



# revision 58
# speedup vs baseline: 1.0932x; 1.0767x over previous
"""Trainium2 Bass kernel for BertClassifierv4 (ragged premise/hypothesis classifier).

Strategy: pure data parallelism. 32 samples are sharded 4-per-core across 8
NeuronCores; all weights are replicated. Host-side numpy does the cheap
index-derived preprocessing (span masks, mean weights, head-padded weight
layouts); the device kernel does all the heavy lifting.

Device-side layout tricks:
  * hsT (bf16, [H, S]) is the canonical operand for every hs @ W matmul
    (PE contracts over partitions).
  * Q/K/V weights are padded per-head from 96 -> 128 so every head lives in
    its own partition tile; row 96 of Q is forced to 1.0 (via bias) and row 96
    of K is overwritten with the premise -1e9 mask, so the scores matmul
    produces masked scores directly in PSUM.
  * Softmax reductions over queries use matmuls with a broadcast [128,1]
    row-scale as lhsT, producing the weighted key-combination already
    broadcast across partitions; a fused DVE tensor_tensor_reduce against
    V^T then yields per-head context columns.
  * All tiny heads (feature extractor, diff/attn/align heads, classifier)
    run once per core batched over the 4 samples.
"""

import os
import sys

import numpy as np

if "/opt/trn_rl_repo" not in sys.path:
    sys.path.insert(0, "/opt/trn_rl_repo")

import ml_dtypes

import concourse.bass as bass
import concourse.bacc as bacc
import concourse.tile as tile
import concourse.tile_sem_assignment as _tsa

# DMA-completion semaphore lanes Tile round-robins over. The historical cap of
# 2 serialized DMA issue (each dma_start waited on the lane's previous user);
# with DMAs now merged into ~35 large transfers the wait-budget pressure that
# motivated the cap is gone.
_tsa.NUM_HWDGE_SEMS = 8
from concourse import mybir
from concourse.bass_utils import run_bass_kernel_spmd
from concourse.masks import make_identity

# Problem constants (hardcoded; kernel.py must be self-contained).
B, S, H = 32, 512, 768
NH, HD = 8, 96
NCLS = 3
SEP = 102
NEG = -1.0e9
NCORES = 8
BL = B // NCORES  # samples per core
HP = 128  # padded head width
HPAD = NH * HP  # 1024
KC = H // 128  # 6 contraction chunks for H
KL = 256  # premise/key range (s1 <= 255)
SQ = S // 128  # 4 seq partition tiles

F32 = mybir.dt.float32
F32R = mybir.dt.float32r
BF16 = mybir.dt.bfloat16

# brow offsets
_BOFF = {}
_off = 0
for _name, _n in [
    ("fe_b1", 512),
    ("fe_b2", 128),
    ("dp_b", 128),
    ("ap_b", 128),
    ("al_b12", 128),
    ("cl_b1", 64),
    ("cl_b2", NCLS),
]:
    _BOFF[_name] = (_off, _n)
    _off += _n
BROW_N = _off


def _build_bass():
    nc = bacc.Bacc(
        "TRN2",
        name="bert_cls_v4",
        num_devices=NCORES,
        use_seq_codegen=os.environ.get("BERT_SEQCG", "0") == "1",
    )

    def din(name, shape, dt):
        return nc.dram_tensor(name, shape, dt, kind="ExternalInput")

    d_hs = din("hs", [BL, S, H], BF16)
    d_hst = din("hst", [BL, H, S], BF16)
    d_wvec = din("wvec", [BL, S, 8], F32)
    d_rows = din("rows", [BL, KL + NH * KL + 2 * S], BF16)
    d_wq = din("wq", [H, HPAD], BF16)
    d_wk = din("wk", [H, HPAD], BF16)
    d_wv = din("wv", [H, HPAD], BF16)
    d_qb = din("qb", [128, NH], F32)
    d_kb = din("kb", [128, NH], F32)
    d_vb = din("vb", [128, NH], F32)
    d_woap = din("woap", [HPAD, 128], BF16)
    d_few1 = din("few1", [H, 512], BF16)
    d_few2 = din("few2", [512, 128], BF16)
    d_dpw = din("dpw", [H, 128], BF16)
    d_alw12 = din("alw12", [2 * H, 128], BF16)
    d_clw1 = din("clw1", [512, 64], BF16)
    d_clw2 = din("clw2", [64, NCLS], BF16)
    d_brow = din("brow", [1, BROW_N], F32)
    d_out = nc.dram_tensor("out", [BL, NCLS], F32, kind="ExternalOutput")

    AF = mybir.ActivationFunctionType
    OP = mybir.AluOpType
    AX = mybir.AxisListType

    with tile.TileContext(nc) as tc:
        with (
            tc.tile_pool(name="consts", bufs=1) as consts,
            tc.tile_pool(name="reps", bufs=1) as reps,
        ):
            # ---- resident weights ----
            def load_merged(pool, dram, rows, cols, dt, tag, eng, nsplit):
                all_t = pool.tile([128, nsplit * cols], dt, tag=tag, name=tag)
                eng.dma_start(
                    all_t[:].rearrange("p (k c) -> p k c", k=nsplit),
                    dram[:, :].rearrange("(k p) c -> p k c", p=128),
                )
                return all_t, [all_t[:, cols * k : cols * (k + 1)] for k in range(nsplit)]

            _, wq_sb = load_merged(consts, d_wq, H, HPAD, BF16, "wqa", nc.sync, KC)
            _, wk_sb = load_merged(consts, d_wk, H, HPAD, BF16, "wka", nc.scalar, KC)
            _, wv_sb = load_merged(consts, d_wv, H, HPAD, BF16, "wva", nc.scalar, KC)
            qb_sb = consts.tile([128, NH], F32, tag="qb", name="qb")
            kb_sb = consts.tile([128, NH], F32, tag="kb", name="kb")
            vb_sb = consts.tile([128, NH], F32, tag="vb", name="vb")
            nc.sync.dma_start(qb_sb[:], d_qb[:, :])
            nc.sync.dma_start(kb_sb[:], d_kb[:, :])
            nc.sync.dma_start(vb_sb[:], d_vb[:, :])
            brow_sb = consts.tile([1, BROW_N], F32, tag="browf", name="browf")
            nc.sync.dma_start(brow_sb[:], d_brow[:, :])
            brow_bf = consts.tile([1, BROW_N], BF16, tag="browb", name="browb")
            nc.vector.tensor_copy(brow_bf[:], brow_sb[:])
            ones1_bf = consts.tile([1, 128], BF16, tag="ones1b", name="ones1b")
            nc.vector.memset(ones1_bf[:], 1.0)
            ones4_f = consts.tile([1, 4], F32, tag="ones4f", name="ones4f")
            nc.vector.memset(ones4_f[:], 1.0)
            ones4_b = consts.tile([1, 4], BF16, tag="ones4b", name="ones4b")
            nc.vector.memset(ones4_b[:], 1.0)
            ident4 = consts.tile([4, 4], F32, tag="id4", name="id4")
            make_identity(nc, ident4[:])

            # ---- persistent per-core representation columns ----
            ATT = reps.tile([128, NH * BL], BF16, tag="ATT", name="ATT")  # col BL*h+i
            ALC = reps.tile([128, 12 * BL], BF16, tag="ALC", name="ALC")  # col BL*c+i
            XFE = reps.tile([128, KC * BL], BF16, tag="XFE", name="XFE")  # col BL*j+i
            SDT = reps.tile([128, KC * BL], BF16, tag="SDT", name="SDT")
            MX = reps.tile([128, KC * BL], F32, tag="MX", name="MX")

            with (
                tc.tile_pool(name="sin", bufs=2) as sin,
                tc.tile_pool(name="sqkv", bufs=3) as sqkv,
                tc.tile_pool(name="sp", bufs=6) as sp,
                tc.tile_pool(name="ssc", bufs=2) as ssc,
                tc.tile_pool(name="pqkv", bufs=1, space="PSUM") as pqkv,
                tc.tile_pool(name="ppss", bufs=2, space="PSUM") as ppss,
                tc.tile_pool(name="ppsw", bufs=1, space="PSUM") as ppsw,
                tc.tile_pool(name="psim", bufs=2, space="PSUM") as psim,
                tc.tile_pool(name="psmall", bufs=1, space="PSUM") as psmall,
            ):
                for i in range(BL):
                    # ---------- loads ----------
                    hst_all = sin.tile([128, KC * S], BF16, tag="hsta", name="hsta")
                    nc.gpsimd.dma_start(
                        hst_all[:, 0 : 3 * S].rearrange("p (k s) -> p k s", k=3),
                        d_hst[i, 0:384, :].rearrange("(k p) s -> p k s", p=128),
                    )
                    nc.gpsimd.dma_start(
                        hst_all[:, 3 * S : 6 * S].rearrange("p (k s) -> p k s", k=3),
                        d_hst[i, 384:768, :].rearrange("(k p) s -> p k s", p=128),
                    )
                    hst_t = [hst_all[:, S * k : S * (k + 1)] for k in range(KC)]
                    hs_all = sin.tile([128, SQ * H], BF16, tag="hsa", name="hsa")
                    nc.gpsimd.dma_start(
                        hs_all[:].rearrange("p (c h) -> p c h", c=SQ),
                        d_hs[i, :, :].rearrange("(c p) h -> p c h", p=128),
                    )
                    hs_t = [hs_all[:, H * c : H * (c + 1)] for c in range(SQ)]
                    rhs6_all = sin.tile([128, SQ * 8], F32, tag="rhs6a", name="rhs6a")
                    nc.sync.dma_start(
                        rhs6_all[:].rearrange("p (c w) -> p c w", c=SQ),
                        d_wvec[i, :, :].rearrange("(c p) w -> p c w", p=128),
                    )
                    rhs6 = [rhs6_all[:, 8 * c : 8 * (c + 1)] for c in range(SQ)]
                    rows_sb = sin.tile([1, KL + NH * KL + 2 * S], BF16, tag="rows", name="rows")
                    nc.sync.dma_start(rows_sb[:], d_rows[i : i + 1, :])
                    pneg_sb = rows_sb[:, 0:KL]
                    pneg8_sb = rows_sb[:, KL : KL + NH * KL]
                    hneg_sb = rows_sb[:, KL + NH * KL : KL + NH * KL + S]
                    aneg_sb = rows_sb[:, KL + NH * KL + S : KL + NH * KL + 2 * S]

                    # ---------- QKV projections (head-padded) ----------
                    qpad = sqkv.tile([128, NH * S], BF16, tag="qpad", name="qpad")
                    kpad = sqkv.tile([128, NH * KL], BF16, tag="kpad", name="kpad")
                    vpad = sqkv.tile([128, NH * KL], BF16, tag="vpad", name="vpad")
                    # premise -1e9 mask into K row 96 (all heads) - independent
                    # of the K copies, which write only rows 0:96
                    nc.sync.dma_start(kpad[96:97, :], pneg8_sb)
                    for h in range(NH):
                        psq = pqkv.tile([128, S], F32, tag="psq", name="psq")
                        for k in range(KC):
                            nc.tensor.matmul(
                                psq[:],
                                lhsT=wq_sb[k][:, 128 * h : 128 * (h + 1)],
                                rhs=hst_t[k][:],
                                start=(k == 0),
                                stop=(k == KC - 1),
                            )
                        nc.scalar.activation(
                            qpad[0:97, S * h : S * (h + 1)],
                            psq[0:97, :],
                            AF.Identity,
                            bias=qb_sb[0:97, h : h + 1],
                        )
                        pskv = pqkv.tile([128, 2 * KL], F32, tag="pskv", name="pskv")
                        for k in range(KC):
                            nc.tensor.matmul(
                                pskv[:, 0:KL],
                                lhsT=wk_sb[k][:, 128 * h : 128 * (h + 1)],
                                rhs=hst_t[k][:, 0:KL],
                                start=(k == 0),
                                stop=(k == KC - 1),
                            )
                        nc.scalar.activation(
                            kpad[0:96, KL * h : KL * (h + 1)],
                            pskv[0:96, 0:KL],
                            AF.Identity,
                            bias=kb_sb[0:96, h : h + 1],
                        )
                    # V deferred: not needed until the context stage, so wv can
                    # arrive on the scalar queue after wk without stalling Q/K
                    for h in range(NH):
                        pskv = pqkv.tile([128, 2 * KL], F32, tag="pskv", name="pskv")
                        for k in range(KC):
                            nc.tensor.matmul(
                                pskv[:, KL : 2 * KL],
                                lhsT=wv_sb[k][:, 128 * h : 128 * (h + 1)],
                                rhs=hst_t[k][:, 0:KL],
                                start=(k == 0),
                                stop=(k == KC - 1),
                            )
                        nc.scalar.activation(
                            vpad[:, KL * h : KL * (h + 1)],
                            pskv[:, KL : 2 * KL],
                            AF.Identity,
                            bias=vb_sb[:, h : h + 1],
                        )

                    # ---------- attention: scores -> exp -> rowscale ----------
                    pat_t = []
                    rs_all = ssc.tile([128, 4 * NH], BF16, tag="rs", name="rs")
                    for t in range(SQ):
                        pat = sp.tile([128, NH * KL], BF16, tag="pat", name="pat")
                        for hh in range(4):
                            pss = ppss.tile([128, 2 * KL], F32, tag="pss", name="pss")
                            for h2 in range(2):
                                h = 2 * hh + h2
                                nc.tensor.matmul(
                                    pss[:, KL * h2 : KL * (h2 + 1)],
                                    lhsT=qpad[0:97, S * h + 128 * t : S * h + 128 * (t + 1)],
                                    rhs=kpad[0:97, KL * h : KL * (h + 1)],
                                    start=True,
                                    stop=True,
                                )
                            nc.scalar.activation(
                                pat[:, 2 * KL * hh : 2 * KL * (hh + 1)], pss[:], AF.Exp
                            )
                        den = ssc.tile([128, NH], F32, tag="den", name="den")
                        nc.vector.tensor_reduce(
                            den[:],
                            pat[:].rearrange("p (h k) -> p h k", h=NH),
                            axis=AX.X,
                            op=OP.add,
                        )
                        invd = ssc.tile([128, NH], F32, tag="invd", name="invd")
                        nc.vector.reciprocal(invd[:], den[:])
                        nc.vector.tensor_scalar(
                            rs_all[:, NH * t : NH * (t + 1)],
                            invd[:],
                            rhs6[t][:, 2:3],
                            None,
                            op0=OP.mult,
                        )
                        pat_t.append(pat)

                    # ---------- attention: weighted key-combination + context ----------
                    for h in range(NH):
                        psw = ppsw.tile([128, KL], F32, tag="psw", name="psw")
                        for t in range(SQ):
                            nc.tensor.matmul(
                                psw[:],
                                lhsT=rs_all[:, NH * t + h : NH * t + h + 1].to_broadcast(
                                    (128, 128)
                                ),
                                rhs=pat_t[t][:, KL * h : KL * (h + 1)],
                                start=(t == 0),
                                stop=(t == SQ - 1),
                            )
                        scr = ssc.tile([128, KL], BF16, tag="scr", name="scr")
                        nc.vector.scalar_tensor_tensor(
                            out=scr[:],
                            in0=vpad[:, KL * h : KL * (h + 1)],
                            scalar=1.0,
                            in1=psw[:],
                            op0=OP.mult,
                            op1=OP.mult,
                            accum_out=ATT[:, BL * h + i : BL * h + i + 1],
                        )

                    # ---------- alignment: p2h (A': rows 0:256, cols 0:512) ----------
                    psxw = psmall.tile([128, 64], F32, tag="psx", name="psx")
                    pswc = psxw[:, 48:56]
                    pa_t = []
                    dena = ssc.tile([128, 2], F32, tag="dena", name="dena")
                    for mt in range(2):
                        psa = psim.tile([128, S], F32, tag="pb", name="pb")
                        for k in range(KC):
                            nc.tensor.matmul(
                                psa[:],
                                lhsT=hst_t[k][:, 128 * mt : 128 * (mt + 1)],
                                rhs=hst_t[k][:],
                                start=(k == 0),
                                stop=False,
                            )
                        nc.tensor.matmul(
                            psa[:],
                            lhsT=ones1_bf[:],
                            rhs=hneg_sb,
                            start=False,
                            stop=True,
                        )
                        # row-max subtraction (sim diagonal ~ ||x||^2 ~ 768 would
                        # overflow exp otherwise)
                        nmax = ssc.tile([128, 1], F32, tag=f"nma{mt}", name=f"nma{mt}")
                        nc.vector.tensor_reduce(
                            nmax[:], psa[:], axis=AX.X, op=OP.max, negate=True
                        )
                        pa = sp.tile([128, S], BF16, tag="pa", name="pa")
                        nc.scalar.activation(
                            pa[:],
                            psa[:],
                            AF.Exp,
                            bias=nmax[:],
                            accum_out=dena[:, mt : mt + 1],
                        )
                        pa_t.append(pa)
                    invda = ssc.tile([128, 2], F32, tag="invda", name="invda")
                    nc.vector.reciprocal(invda[:], dena[:])
                    rsa = []
                    for mt in range(2):
                        r = ssc.tile([128, 1], BF16, tag=f"rsa{mt}", name=f"rsa{mt}")
                        nc.vector.tensor_scalar(
                            r[:],
                            invda[:, mt : mt + 1],
                            rhs6[mt][:, 1:2],
                            None,
                            op0=OP.mult,
                        )
                        rsa.append(r)
                    for tb in range(4):
                        for mt in range(2):
                            nc.tensor.matmul(
                                pswc[:, tb : tb + 1],
                                lhsT=pa_t[mt][:, 128 * tb : 128 * (tb + 1)],
                                rhs=rsa[mt][:],
                                start=(mt == 0),
                                stop=(mt == 1),
                            )

                    # ---------- alignment: h2p (B': rows 0:512, cols 0:256) ----------
                    pb_t = []
                    denb = ssc.tile([128, 4], F32, tag="denb", name="denb")
                    for mt in range(SQ):
                        psb = psim.tile([128, KL], F32, tag="pb", name="pb")
                        for k in range(KC):
                            nc.tensor.matmul(
                                psb[:],
                                lhsT=hst_t[k][:, 128 * mt : 128 * (mt + 1)],
                                rhs=hst_t[k][:, 0:KL],
                                start=(k == 0),
                                stop=False,
                            )
                        nc.tensor.matmul(
                            psb[:],
                            lhsT=ones1_bf[:],
                            rhs=pneg_sb,
                            start=False,
                            stop=True,
                        )

                        nmax = ssc.tile([128, 1], F32, tag=f"nmb{mt}", name=f"nmb{mt}")
                        nc.vector.tensor_reduce(
                            nmax[:], psb[:], axis=AX.X, op=OP.max, negate=True
                        )
                        pb = sp.tile([128, KL], BF16, tag="pbt", name="pbt")
                        nc.scalar.activation(
                            pb[:],
                            psb[:],
                            AF.Exp,
                            bias=nmax[:],
                            accum_out=denb[:, mt : mt + 1],
                        )
                        pb_t.append(pb)
                    invdb = ssc.tile([128, 4], F32, tag="invdb", name="invdb")
                    nc.vector.reciprocal(invdb[:], denb[:])
                    rsb = []
                    for mt in range(SQ):
                        r = ssc.tile([128, 1], BF16, tag=f"rsb{mt}", name=f"rsb{mt}")
                        nc.vector.tensor_scalar(
                            r[:],
                            invdb[:, mt : mt + 1],
                            rhs6[mt][:, 2:3],
                            None,
                            op0=OP.mult,
                        )
                        rsb.append(r)
                    for tb in range(2):
                        for mt in range(SQ):
                            nc.tensor.matmul(
                                pswc[:, 4 + tb : 5 + tb],
                                lhsT=pb_t[mt][:, 128 * tb : 128 * (tb + 1)],
                                rhs=rsb[mt][:],
                                start=(mt == 0),
                                stop=(mt == SQ - 1),
                            )
                    # move alignment combination vectors into the x6 rhs columns
                    for c in range(SQ):
                        nc.vector.tensor_copy(rhs6[c][:, 4:5], pswc[:, c : c + 1])
                    for c in range(2):
                        nc.vector.tensor_copy(rhs6[c][:, 5:6], pswc[:, 4 + c : 5 + c])
                    # bf16 copy of the 6 weight columns (matmul rhs must match
                    # the bf16 hs_t lhsT dtype)
                    rhs6b = []
                    for c in range(SQ):
                        t = ssc.tile([128, 6], BF16, tag=f"rhs6b{c}", name=f"rhs6b{c}")
                        nc.vector.tensor_copy(t[:], rhs6[c][:, 0:6])
                        rhs6b.append(t)

                    # ---------- masked max over sequence (per d-chunk) ----------
                    psneg = ppss.tile([128, 2 * KL], F32, tag="pss", name="pss")
                    nc.tensor.matmul(
                        psneg[:, 0:S], lhsT=ones1_bf[:], rhs=aneg_sb, start=True, stop=True
                    )
                    for k in range(KC):
                        scr2 = ssc.tile([128, S], F32, tag="scr2", name="scr2")
                        nc.vector.scalar_tensor_tensor(
                            out=scr2[:],
                            in0=hst_t[k][:],
                            scalar=0.0,
                            in1=psneg[:, 0:S],
                            op0=OP.add,
                            op1=OP.add,
                        )
                        nc.vector.tensor_reduce(
                            MX[:, BL * k + i : BL * k + i + 1],
                            scr2[:],
                            axis=AX.X,
                            op=OP.max,
                        )

                    # ---------- x6 matvec: [mean, prem-mean, hyp-mean, pooled, al1, al2] ----------
                    psx = psxw[:, 0:48]
                    for j in range(KC):
                        for c in range(SQ):
                            nc.tensor.matmul(
                                psxw[:, 8 * j : 8 * j + 6],
                                lhsT=hs_t[c][:, 128 * j : 128 * (j + 1)],
                                rhs=rhs6b[c][:],
                                start=(c == 0),
                                stop=(c == SQ - 1),
                            )
                    # stage PSUM x6 result through SBUF (DVE can read only one
                    # PSUM operand per instruction); copy only written columns
                    x6sb = ssc.tile([128, 36], F32, tag="x6sb", name="x6sb")
                    nc.vector.tensor_copy(
                        x6sb[:].rearrange("p (g c) -> p g c", g=KC),
                        psx.rearrange("p (g c) -> p g c", g=KC)[:, :, 0:6],
                    )
                    # strided views: cols i, i+BL, ... (count KC, step BL)
                    xfe_cols = XFE[:, i::BL]
                    sdt_cols = SDT[:, i::BL]
                    mx_cols = MX[:, i::BL]
                    mean_cols = x6sb[:, 0::6]
                    prem_cols = x6sb[:, 1::6]
                    hyp_cols = x6sb[:, 2::6]
                    pool_cols = x6sb[:, 3::6]
                    al1_cols = x6sb[:, 4::6]
                    al2_cols = x6sb[:, 5::6]
                    tmp6 = ssc.tile([128, KC], F32, tag="tmp6", name="tmp6")
                    nc.vector.tensor_add(tmp6[:], mean_cols, pool_cols)
                    nc.vector.tensor_add(xfe_cols, tmp6[:], mx_cols)
                    tmp7 = ssc.tile([128, KC], F32, tag="tmp7", name="tmp7")
                    nc.vector.tensor_sub(tmp7[:], prem_cols, hyp_cols)
                    nc.scalar.activation(sdt_cols, tmp7[:], AF.Abs)
                    alc1_cols = ALC[:, i : BL * KC : BL]
                    alc2_cols = ALC[:, BL * KC + i :: BL]
                    nc.vector.tensor_copy(alc1_cols, al1_cols)
                    nc.vector.tensor_copy(alc2_cols, al2_cols)

            # head-phase weights: issued after the sample loop so their DMAs
            # don't delay the first sample's input loads at startup
            _, woap_sb = load_merged(consts, d_woap, HPAD, 128, BF16, "woapa", nc.sync, 8)
            _, few1_sb = load_merged(consts, d_few1, H, 512, BF16, "few1a", nc.sync, KC)
            _, few2_sb = load_merged(consts, d_few2, 512, 128, BF16, "few2a", nc.sync, 4)
            _, dpw_sb = load_merged(consts, d_dpw, H, 128, BF16, "dpwa", nc.sync, KC)
            _, alw12_sb = load_merged(consts, d_alw12, 2 * H, 128, BF16, "alw12a", nc.sync, 12)
            _, clw1_sb = load_merged(consts, d_clw1, 512, 64, BF16, "clw1a", nc.sync, 4)
            clw2_sb = consts.tile([64, NCLS], BF16, tag="clw2", name="clw2")
            nc.sync.dma_start(clw2_sb[:], d_clw2[:, :])

            # ---------- per-core head phase (batched over BL samples) ----------
            with (
                tc.tile_pool(name="shead", bufs=2) as sh,
                tc.tile_pool(name="phead", bufs=1, space="PSUM") as ph,
                tc.tile_pool(name="ptr", bufs=2, space="PSUM") as ptr,
            ):
                def brow_f(name):
                    o, n = _BOFF[name]
                    return brow_sb[:, o : o + n]

                def brow_b(name):
                    o, n = _BOFF[name]
                    return brow_bf[:, o : o + n]

                # Z assembly [BL, 512]: feat | diff | attn | align
                psZ = ph.tile([BL, 512], F32, tag="psZ", name="psZ")
                for j in range(KC):
                    nc.tensor.matmul(
                        psZ[:, 128:256],
                        lhsT=SDT[:, BL * j : BL * (j + 1)],
                        rhs=dpw_sb[j][:],
                        start=(j == 0),
                        stop=False,
                    )
                nc.tensor.matmul(
                    psZ[:, 128:256], lhsT=ones4_b[:], rhs=brow_b("dp_b"),
                    start=False, stop=True,
                )
                for c in range(8):
                    nc.tensor.matmul(
                        psZ[:, 256:384],
                        lhsT=ATT[:, BL * c : BL * (c + 1)],
                        rhs=woap_sb[c][:],
                        start=(c == 0),
                        stop=False,
                    )
                nc.tensor.matmul(
                    psZ[:, 256:384], lhsT=ones4_b[:], rhs=brow_b("ap_b"),
                    start=False, stop=True,
                )
                for c in range(12):
                    nc.tensor.matmul(
                        psZ[:, 384:512],
                        lhsT=ALC[:, BL * c : BL * (c + 1)],
                        rhs=alw12_sb[c][:],
                        start=(c == 0),
                        stop=False,
                    )
                nc.tensor.matmul(
                    psZ[:, 384:512], lhsT=ones4_b[:], rhs=brow_b("al_b12"),
                    start=False, stop=True,
                )
                # feature extractor first layer + layernorm
                psz1 = ph.tile([BL, 512], F32, tag="psz1", name="psz1")
                for j in range(KC):
                    nc.tensor.matmul(
                        psz1[:],
                        lhsT=XFE[:, BL * j : BL * (j + 1)],
                        rhs=few1_sb[j][:],
                        start=(j == 0),
                        stop=False,
                    )
                nc.tensor.matmul(
                    psz1[:], lhsT=ones4_b[:], rhs=brow_b("fe_b1"), start=False, stop=True
                )
                musum = sh.tile([BL, 1], F32, tag="musum", name="musum")
                nc.vector.tensor_reduce(musum[:], psz1[:], axis=AX.X, op=OP.add)
                mu = sh.tile([BL, 1], F32, tag="mu", name="mu")
                nc.vector.tensor_scalar(mu[:], musum[:], 1.0 / 512, None, op0=OP.mult)
                hc = sh.tile([BL, 512], F32, tag="hc", name="hc")
                nc.vector.tensor_scalar(hc[:], psz1[:], mu[:], None, op0=OP.subtract)
                sq = sh.tile([BL, 512], F32, tag="sq", name="sq")
                ssum = sh.tile([BL, 1], F32, tag="ssum", name="ssum")
                nc.vector.scalar_tensor_tensor(
                    out=sq[:],
                    in0=hc[:],
                    scalar=1.0,
                    in1=hc[:],
                    op0=OP.mult,
                    op1=OP.mult,
                    accum_out=ssum[:],
                )
                varv = sh.tile([BL, 1], F32, tag="varv", name="varv")
                nc.vector.tensor_scalar(
                    varv[:], ssum[:], 1.0 / 512, 1.0e-5, op0=OP.mult, op1=OP.add
                )
                lnv = sh.tile([BL, 1], F32, tag="lnv", name="lnv")
                nc.scalar.activation(lnv[:], varv[:], AF.Ln)
                rstd = sh.tile([BL, 1], F32, tag="rstd", name="rstd")
                nc.scalar.activation(rstd[:], lnv[:], AF.Exp, scale=-0.5)
                hn = sh.tile([BL, 512], F32, tag="hn", name="hn")
                nc.vector.tensor_scalar(hn[:], hc[:], rstd[:], None, op0=OP.mult)
                # transpose hn -> columns
                hnc = sh.tile([128, 4 * BL], BF16, tag="hnc", name="hnc")
                for c in range(4):
                    pt = ptr.tile([128, BL], F32, tag="pt", name="pt")
                    nc.tensor.transpose(pt[:], hn[:, 128 * c : 128 * (c + 1)], ident4[:])
                    nc.vector.tensor_copy(hnc[:, BL * c : BL * (c + 1)], pt[:])

                for c in range(4):
                    nc.tensor.matmul(
                        psZ[:, 0:128],
                        lhsT=hnc[:, BL * c : BL * (c + 1)],
                        rhs=few2_sb[c][:],
                        start=(c == 0),
                        stop=False,
                    )
                nc.tensor.matmul(
                    psZ[:, 0:128], lhsT=ones4_b[:], rhs=brow_b("fe_b2"),
                    start=False, stop=True,
                )
                eZ = sh.tile([BL, 512], F32, tag="eZ", name="eZ")
                nc.scalar.activation(eZ[:], psZ[:], AF.Exp)
                tZ = sh.tile([BL, 512], F32, tag="tZ", name="tZ")
                nc.scalar.activation(tZ[:], eZ[:], AF.Tanh)
                comb = sh.tile([BL, 512], F32, tag="comb", name="comb")
                nc.vector.tensor_mul(comb[:], psZ[:], tZ[:])
                cbc = sh.tile([128, 4 * BL], BF16, tag="cbc", name="cbc")
                for c in range(4):
                    pt = ptr.tile([128, BL], F32, tag="pt", name="pt")
                    nc.tensor.transpose(pt[:], comb[:, 128 * c : 128 * (c + 1)], ident4[:])
                    nc.vector.tensor_copy(cbc[:, BL * c : BL * (c + 1)], pt[:])

                # classifier
                psz2 = ph.tile([BL, 64], F32, tag="psz2", name="psz2")
                for c in range(4):
                    nc.tensor.matmul(
                        psz2[:],
                        lhsT=cbc[:, BL * c : BL * (c + 1)],
                        rhs=clw1_sb[c][:],
                        start=(c == 0),
                        stop=False,
                    )
                nc.tensor.matmul(
                    psz2[:], lhsT=ones4_b[:], rhs=brow_b("cl_b1"), start=False, stop=True
                )
                eu = sh.tile([BL, 64], F32, tag="eu", name="eu")
                nc.scalar.activation(eu[:], psz2[:], AF.Exp)
                tu = sh.tile([BL, 64], F32, tag="tu", name="tu")
                nc.scalar.activation(tu[:], eu[:], AF.Tanh)
                uu = sh.tile([BL, 64], F32, tag="uu", name="uu")
                nc.vector.tensor_mul(uu[:], psz2[:], tu[:])
                ptu = ptr.tile([64, BL], F32, tag="pt", name="pt")
                nc.tensor.transpose(ptu[:], uu[:], ident4[:])
                uc = sh.tile([64, BL], BF16, tag="uc", name="uc")
                nc.vector.tensor_copy(uc[:], ptu[:])
                pslog = ph.tile([BL, NCLS], F32, tag="pslog", name="pslog")
                nc.tensor.matmul(pslog[:], lhsT=uc[:], rhs=clw2_sb[:], start=True, stop=False)
                nc.tensor.matmul(
                    pslog[:], lhsT=ones4_b[:], rhs=brow_b("cl_b2"), start=False, stop=True
                )
                out_sb = sh.tile([BL, NCLS], F32, tag="outsb", name="outsb")
                nc.vector.tensor_copy(out_sb[:], pslog[:])
                nc.sync.dma_start(d_out[:, :], out_sb[:])

    nc.compile()
    return nc


def _host_prep(inputs):
    """Compute per-core input maps from the full problem inputs."""
    f32 = np.float32
    bf16 = ml_dtypes.bfloat16
    hs = np.asarray(inputs["hidden_states"], dtype=f32)
    ids = np.asarray(inputs["input_ids"])
    am = np.asarray(inputs["attention_mask"]).astype(f32)

    sep = ids == SEP
    s1 = np.argmax(sep, axis=1)
    s2 = (S - 1) - np.argmax(sep[:, ::-1], axis=1)
    pos = np.arange(S)[None, :]
    prem = ((pos >= 1) & (pos < s1[:, None])).astype(f32)
    hyp = ((pos > s1[:, None]) & (pos < s2[:, None])).astype(f32)

    def wnorm(m):
        return m / np.clip(m.sum(1, keepdims=True), 1e-9, None)

    amw = wnorm(am)
    premw = wnorm(prem)
    hypw = wnorm(hyp)
    wvec = np.zeros((B, S, 8), dtype=f32)
    wvec[:, :, 0] = amw
    wvec[:, :, 1] = premw
    wvec[:, :, 2] = hypw
    wvec[:, 0, 3] = 1.0  # e0 -> pooled
    wvec[:, :, 6] = np.where(prem > 0, 0.0, NEG)  # premise row-mask for simA
    pneg = np.where(prem[:, :KL] > 0, 0.0, NEG).astype(bf16)
    hneg = np.where(hyp > 0, 0.0, NEG).astype(bf16)
    aneg = np.where(am > 0, 0.0, NEG).astype(bf16)

    hstf = np.ascontiguousarray(hs.transpose(0, 2, 1))
    hst = hstf.astype(bf16)
    hsb = hs.astype(bf16)

    def padw(w, b, scale=1.0, row96=0.0):
        w = np.asarray(w, dtype=f32) * scale
        b = np.asarray(b, dtype=f32) * scale
        wp = np.zeros((H, HPAD), dtype=f32)
        bp = np.zeros((128, NH), dtype=f32)
        for h in range(NH):
            wp[:, HP * h : HP * h + HD] = w[:, HD * h : HD * (h + 1)]
            bp[0:HD, h] = b[HD * h : HD * (h + 1)]
            bp[HD, h] = row96
        return wp.astype(bf16), bp

    isq = 1.0 / np.sqrt(np.float32(HD))
    wq_p, qb = padw(inputs["mha_wq"], inputs["mha_bq"], scale=isq, row96=1.0)
    wk_p, kb = padw(inputs["mha_wk"], inputs["mha_bk"], scale=1.0, row96=0.0)
    wv_p, vb = padw(inputs["mha_wv"], inputs["mha_bv"], scale=1.0, row96=0.0)

    wo = np.asarray(inputs["mha_wo"], dtype=f32)
    bo = np.asarray(inputs["mha_bo"], dtype=f32)
    ap_w = np.asarray(inputs["ap_w"], dtype=f32)
    ap_b = np.asarray(inputs["ap_b"], dtype=f32)
    woap768 = wo @ ap_w  # [768, 128]
    woap = np.zeros((HPAD, 128), dtype=f32)
    for h in range(NH):
        woap[HP * h : HP * h + HD, :] = woap768[HD * h : HD * (h + 1), :]
    ap_b_eff = bo @ ap_w + ap_b

    fe_w1 = np.asarray(inputs["fe_w1"], dtype=f32)
    fe_g = np.asarray(inputs["fe_g"], dtype=f32)
    fe_be = np.asarray(inputs["fe_be"], dtype=f32)
    fe_w2 = np.asarray(inputs["fe_w2"], dtype=f32)
    fe_b2 = np.asarray(inputs["fe_b2"], dtype=f32)
    # LN(h)*g + be then @ fe_w2 + fe_b2  ==  LNraw(h) @ (g*fe_w2) + (be@fe_w2 + fe_b2)
    few2 = fe_w2 * fe_g[:, None]
    fe_b2_eff = fe_be @ fe_w2 + fe_b2

    brow = np.zeros((1, BROW_N), dtype=f32)

    def setb(name, v):
        o, n = _BOFF[name]
        brow[0, o : o + n] = v

    setb("fe_b1", np.asarray(inputs["fe_b1"], dtype=f32))
    setb("fe_b2", fe_b2_eff)
    setb("dp_b", np.asarray(inputs["dp_b"], dtype=f32))
    setb("ap_b", ap_b_eff)
    al_w1 = np.asarray(inputs["al_w1"], dtype=f32)
    al_w2 = np.asarray(inputs["al_w2"], dtype=f32)
    al_b1 = np.asarray(inputs["al_b1"], dtype=f32)
    al_b2 = np.asarray(inputs["al_b2"], dtype=f32)
    setb("al_b12", al_b1 @ al_w2 + al_b2)
    setb("cl_b1", np.asarray(inputs["cl_b1"], dtype=f32))
    setb("cl_b2", np.asarray(inputs["cl_b2"], dtype=f32))

    shared = dict(
        wq=wq_p, wk=wk_p, wv=wv_p, qb=qb, kb=kb, vb=vb,
        woap=woap.astype(bf16),
        few1=fe_w1.astype(bf16),
        few2=few2.astype(bf16),
        dpw=np.asarray(inputs["dp_w"], dtype=f32).astype(bf16),
        alw12=(al_w1 @ al_w2).astype(bf16),
        clw1=np.asarray(inputs["cl_w1"], dtype=f32).astype(bf16),
        clw2=np.asarray(inputs["cl_w2"], dtype=f32).astype(bf16),
        brow=brow,
    )
    in_maps = []
    for core in range(NCORES):
        sl = slice(core * BL, (core + 1) * BL)
        m = dict(shared)
        m["hs"] = np.ascontiguousarray(hsb[sl])
        m["hst"] = np.ascontiguousarray(hst[sl])
        m["wvec"] = np.ascontiguousarray(wvec[sl])
        m["rows"] = np.ascontiguousarray(
            np.concatenate(
                [pneg[sl], np.tile(pneg[sl], (1, NH)), hneg[sl], aneg[sl]], axis=1
            )
        )
        in_maps.append(m)
    return in_maps


_NC_CACHE = {}


class _Exec:
    """Cached PJRT executable over the 8 axon-tunneled cores (mirrors
    bass2jax.run_bass_via_pjrt's multi-core path, but reusable so repeat
    calls don't re-trace/re-compile)."""

    def __init__(self):
        import jax
        import concourse.bass2jax as b2j
        from jax.experimental.shard_map import shard_map
        from jax.sharding import Mesh, PartitionSpec

        self.jax = jax
        self.b2j = b2j
        nc = _build_bass()
        self.nc = nc
        b2j.install_neuronx_cc_hook()
        in_names, out_names, out_avals = [], [], []
        partition_name = (
            nc.partition_id_tensor.name if nc.partition_id_tensor else None
        )
        for alloc in nc.m.functions[0].allocations:
            if not isinstance(alloc, mybir.MemoryLocationSet):
                continue
            name = alloc.memorylocations[0].name
            if alloc.kind == "ExternalInput":
                if name != partition_name:
                    in_names.append(name)
            elif alloc.kind == "ExternalOutput":
                out_names.append(name)
                out_avals.append(
                    jax.core.ShapedArray(
                        tuple(alloc.tensor_shape), mybir.dt.np(alloc.dtype)
                    )
                )
        self.in_names = list(in_names)
        self.out_names = list(out_names)
        self.out_avals = out_avals
        n_params = len(in_names)
        n_outs = len(out_avals)
        all_in_names = list(in_names) + list(out_names)
        if partition_name is not None:
            all_in_names.append(partition_name)
        donate = tuple(range(n_params, n_params + n_outs))

        def _body(*args):
            operands = list(args)
            if partition_name is not None:
                operands.append(b2j.partition_id_tensor())
            outs = b2j._bass_exec_p.bind(
                *operands,
                out_avals=tuple(out_avals),
                in_names=tuple(all_in_names),
                out_names=tuple(out_names),
                lowering_input_output_aliases=(),
                sim_require_finite=True,
                sim_require_nnan=True,
                nc=nc,
            )
            return tuple(outs)

        devices = jax.devices()[:NCORES]
        mesh = Mesh(np.asarray(devices), ("core",))
        in_specs = (PartitionSpec("core"),) * (n_params + n_outs)
        out_specs = (PartitionSpec("core"),) * n_outs
        self.sharded = jax.jit(
            shard_map(
                _body,
                mesh=mesh,
                in_specs=in_specs,
                out_specs=out_specs,
                check_rep=False,
            ),
            donate_argnums=donate,
            keep_unused=True,
        )

    def concat_inputs(self, in_maps):
        return [
            np.concatenate([m[name] for m in in_maps], axis=0)
            for name in self.in_names
        ]

    def zeros(self):
        return [
            np.zeros((NCORES * a.shape[0], *a.shape[1:]), a.dtype)
            for a in self.out_avals
        ]

    def run(self, concat_in):
        out_arrs = self.sharded(*concat_in, *self.zeros())
        return [np.asarray(o) for o in out_arrs]


def _get_exec():
    if "exec" not in _NC_CACHE:
        _NC_CACHE["exec"] = _Exec()
    return _NC_CACHE["exec"]


def _run_coresim(in_maps):
    """Fallback executor: run each core's shard through CoreSim (slow but
    exact) if the PJRT/hardware path is unavailable."""
    from concourse.bass_interp import CoreSim

    if "exec" in _NC_CACHE:
        nc = _NC_CACHE["exec"].nc
    elif "nc" in _NC_CACHE:
        nc = _NC_CACHE["nc"]
    else:
        nc = _NC_CACHE["nc"] = _build_bass()
    outs = []
    for m in in_maps:
        sim = CoreSim(nc, require_finite=False, require_nnan=False)
        for name, val in m.items():
            sim.tensor(name)[:] = val
        sim.simulate()
        outs.append(np.array(sim.tensor("out")))
    return np.concatenate(outs, axis=0)


def kernel(**inputs):
    in_maps = _host_prep(inputs)
    try:
        ex = _get_exec()
        concat_in = ex.concat_inputs(in_maps)
        outs = ex.run(concat_in)
        out = outs[ex.out_names.index("out")].reshape(B, NCLS)
    except Exception:
        out = _run_coresim(in_maps)
    return np.ascontiguousarray(out.astype(np.float32).reshape(B, NCLS))

